# revision 61
# baseline (speedup 1.0000x reference)
"""Multi-head causal attention (B=2, S=2048, D=1024, 16 heads x 64) on 8 trn2
NeuronCores.

Sharding: core c = 4*b + g handles batch b and heads [4g, 4g+4) (tensor
parallel over heads, data parallel over batch). Each core:
  - projects q/k/v for its heads from x[b] (wqkv column-sharded by head),
  - applies rotary embeddings,
  - computes causal softmax(q k^T / sqrt(d)) v in a transposed-score layout,
  - multiplies by its shard of wo^T to produce a partial [D, S] output (fp16).
The host sums the 4 head-group partials per batch and transposes.

Device-side layouts (per core):
  xt      [128, 4, 8, 512]  x[b]^T, s-chunk-major: [partition, s-chunk,
                          k-tile, 512 queries] so each 1MB s-chunk is one
                          contiguous DMA and window 0's projection does not
                          wait for the full 4MB load (~100-110GB/s per queue)
  wqkt    [128, 2, 8, 256]  W_{q,k}^T as [partition, q/k, k-tile, 4 heads x
                          (32 evens | 32 odds)] so RoPE runs as full-width
                          vector ops and q/k halves are contiguous DMAs
  wvt     [128, 8, 256]   W_v^T, natural head-dim order
  wot     [128, 2, 1024]  wo[:, head cols]^T (matmul stationary)
  cosA/sinA [128, 2048]   rotary tables tiled 4x over the 32 pair dims
  tril2   [128, 256]      upper-triangular 0/1 x2 (valid = key <= query);
                          cast once on device to tril8 (fp8) for the fp8 at
  qhat/khat [th][128, S]  packed head pairs: rows 64j..64j+64 = head 2th+j
                          as [evens(32); odds(32)], bf16
  v8_sb   [128,16,4,80]   fp8 v, per s-tile per head slot [v(64)|ones|pad];
                          the 80-slot keeps the DoubleRow weights AP slab
                          stride (4*80) a multiple of 16 bytes
  vb_sb   [128,2,4,65]    exact bf16 v for s-tiles 0,1 only
  at      [128,2,2,512]   exp output per (head pair, K-TILE PAIR): fp8, dims
                          [key, k-tile slab, head, query]
  outp    [8, 128, 2048]  partial output, d-major, fp16

Precision strategy (metric = max|err|/absmax ~ 0.75x rms_rel; budget 2e-2):
q/k/v projections, rope, scores and wo all stay bf16 - fp8 anywhere on
those paths puts ~5-10% relative noise on y (softmax does NOT attenuate
relative error: y shrinks with attention entropy exactly as fast as the
noise). The ONE fp8 win that survives: the AV contraction, as
perf_mode=DoubleRow over K=256 (two 128-key tiles packed in the at tile's
slab dim, v8 slab stride 320B), HW-measured at 2x bf16 throughput. Early
queries (low entropy, y near full variance, dominate the max-err metric)
are protected by running window 0's first k-tile pair through an exact
bf16 two-matmul AV (queries 0..255 then see zero fp8 noise). exp runs
with bias=-2 so e^x stays inside fp8e4's 240 max; the bias cancels in the
softmax ratio since the ones-row denominator sees the same factor.
Measured rel_err 4.1e-3 (bf16 baseline was 3.8e-3).

Scores use K=64 matmuls (tile_position row groups 0/64) so the two heads
of a pair run concurrently on the PE array halves; each (pair, k-tile)
produces a two-head-wide [128, 2x512] PSUM tile consumed by ONE wide exp
ACTIVATE writing the fp8 at slab. ACTIVATE cost is free-size x 0.83ns
regardless of dtype, so keys stay on all 128 partitions and the exp
stream (~65-80us total) is the hard scalar-engine floor. For a diagonal
k-tile pair the odd slab's pre-qs query range is never written by its
exp but is summed by the DR matmul: a gpsimd memset zeroes the gap
(pool rotation leaves stale data there, not zeros).

Schedule: both sc1 projection pairs are front-loaded; anchored warm
matmuls (rhs pinned on already-loaded data so the Tile scheduler cannot
float them ahead) bridge the HBM-bound 14-26us startup window - the first
20us of input DMA is at the 2x ~105GB/s HWDGE roofline, so xt0/wqk/cos/
sin/xt1 split need-ordered across the sync+scalar queues (xt1 half on
each), and xt2/xt3/wot triggers are emitted LATE (mid-pipeline) so their
bulk never sits ahead of the latency-critical rope-shuffle DMAs. gpsimd
(SWDGE, ~3x slower, software FIFO) carries no bulk input and loses the
output stores from chunk >= 1 (its backlog otherwise delays the final
drain by ~5us). The scalar queue carries ONLY the exp stream plus
startup/late input triggers.

The per-pair softmax normalize is SPLIT AND DEFERRED to keep the in-order
DVE/gpsimd queues from head-of-line-blocking the AV critical path
(semaphore waits transitively stall everything behind them in a queue):
psum evict + fp32 reciprocal emit at pair end; the gpsimd partition-
broadcast of 1/z emits at the NEXT pair's m==nm-2 (after its fills'
shuffles are queued); the yt multiplies + yst partition-shift DMA emit
TWO pairs later (their broadcast is then guaranteed complete, so they
enter the DVE queue wait-free). Everything pending flushes before the
wo(2) interleave in the last pair (which reads yt window 2) and again at
the tail. Rope ops stage the projection psum to bf16 first: the six
rotate ops then run in DVE 2x mode and the pp psum banks free after two
copies instead of four reads. wo output chunks stream as per-dt slivers
through the pre_k hooks of later attention passes; the final chunk
borrows the idle psY banks so four po tiles rotate (PE streams through
the copies at full pstate) and its stores ride sync+scalar only. Tail
warm matmuls are staged on successively later anchors (last at tile ->
yt -> ytu -> zr) so the PE never idles >2us through the final normalize
chain and the last wo runs at speed.

Beware the power-state lottery: the chip drops ALL engine clocks ~17%
(PE 2.4->2.0GHz) under sustained load, stickily across runs (PE busy in
the trace inflates 128->152us with unchanged instruction count). Verify
the clock via back-to-back N=512 matmul deltas (216ns warm vs 259ns)
before comparing timings. Measured fast-state: 163.4us (bf16 baseline
191.0us); do not trust single-run deltas under ~5us.

Things tried that did NOT work (traced, reverted):
  - fp8 DoubleRow for projections/scores: numerically dead (see above).
  - tril mask multiplies on gpsimd: its FIFO serialized the AV path
    behind broadcasts/shuffles (12-15us window gaps).
  - on-device cos/sin 4x partition dup via SBUF-SBUF DMA: correct in
    isolation, corrupts under load in-kernel (suspected partition-region
    dependency tracking); host-tiled [128,S] kept instead.
  - scores as 4 concurrent 32-row DoubleRow matmuls: AP base_partition
    is ISA-limited to {0,32,64}; row 96 is unreachable.
"""

import numpy as np
import ml_dtypes

import concourse.bass as bass
import concourse.mybir as mybir
import concourse.tile as tile
from concourse import bacc
from concourse.bass_utils import run_bass_kernel_spmd

N_CORES = 8
B, S, DIM = 2, 2048, 1024
N_HEAD, HD = 16, 64
HPC = N_HEAD // 4  # heads per core = 4
KT = DIM // 128  # 8 contraction tiles over model dim
F32 = mybir.dt.float32
F16 = mybir.dt.float16
BF16 = mybir.dt.bfloat16
FP8 = mybir.dt.float8e4
DR = mybir.MatmulPerfMode.DoubleRow
MM_DT = BF16
W = 512  # query window width
NW = S // W  # 4 windows
VSLOT = 80   # fp8 v columns per head slot ([v(64) | ones | pad]; 16B-aligned
             # so the DoubleRow weights AP slab stride (4*VSLOT) is %16)
VTILE = HPC * VSLOT  # 320 fp8 v columns per s-tile
EXP_BIAS = -2.0  # logits bias before exp: keeps e^x inside fp8e4 range
                 # (max finite 240); cancels exactly in the softmax ratio

_programs = {}


def _np_mm_dt(md):
    return ml_dtypes.bfloat16 if md == BF16 else np.float32


def _build_program(causal: bool, md=MM_DT):
    nc = bacc.Bacc("TRN2", target_bir_lowering=False, debug=False,
                   num_devices=N_CORES)

    xt_d = nc.dram_tensor("xt", [128, 4, KT, 512], md, kind="ExternalInput")
    wqkt_d = nc.dram_tensor("wqkt", [128, 2, KT, 256], md, kind="ExternalInput")
    wvt_d = nc.dram_tensor("wvt", [128, KT, 256], md, kind="ExternalInput")
    wot_d = nc.dram_tensor("wot", [128, 2, 1024], md, kind="ExternalInput")
    cos_d = nc.dram_tensor("cosA", [128, S], md, kind="ExternalInput")
    sin_d = nc.dram_tensor("sinA", [128, S], md, kind="ExternalInput")
    tril_d = nc.dram_tensor("tril2", [128, 256], md, kind="ExternalInput")
    out_d = nc.dram_tensor("outp", [KT, 128, S], F16, kind="ExternalOutput")

    with tile.TileContext(nc) as tc:
      with (
        tc.tile_pool(name="persist", bufs=1) as persist,
        tc.tile_pool(name="pha", bufs=1) as pha,
        tc.tile_pool(name="rope_out", bufs=4) as rope_out,
        tc.tile_pool(name="rope_tmp", bufs=3) as rope_tmp,
        tc.tile_pool(name="attn", bufs=4) as attn_pool,
        tc.tile_pool(name="attnb", bufs=2) as attnb_pool,
        tc.tile_pool(name="norm", bufs=3) as norm_pool,
        tc.tile_pool(name="ystage", bufs=2) as ystage,
        tc.tile_pool(name="ostage", bufs=3) as ostage,
        tc.tile_pool(name="psS", bufs=2, space="PSUM") as psS,
        tc.tile_pool(name="psY", bufs=1, space="PSUM") as psY,
        tc.tile_pool(name="pp", bufs=1, space="PSUM") as pp,
      ):
         # packed head-pair tiles: rows 64j.. = head 2th+j as [E32; O32]
         qhat = [persist.tile([128, S], md, tag=f"qhat{t}", name=f"qhat{t}") for t in range(2)]
         khat = [persist.tile([128, S], md, tag=f"khat{t}", name=f"khat{t}") for t in range(2)]
         # fp8 v for the DoubleRow AV path: 16 s-tiles x 4 slots of
         # [v(64) | ones | pad(15)]
         v8_sb = persist.tile([128, 16, HPC, VSLOT], FP8, tag="v8_sb")
         # exact bf16 v for s-tiles 0,1 (window-0 first k-pair runs bf16 so
         # queries 0..255 see no fp8 noise; their y is near full variance and
         # dominates the max-err metric)
         vb_sb = persist.tile([128, 2, HPC, HD + 1], md, tag="vb_sb")
         yt_sb = [persist.tile([128, S], md, tag=f"yt{t}", name=f"yt{t}") for t in range(2)]
         tril_sb = persist.tile([128, 256], md, tag="tril")
         tril8 = persist.tile([128, 256], FP8, tag="tril8")
         nbias = persist.tile([128, 1], F32, tag="nbias")
         wot = persist.tile([128, 2, 1024], md, tag="wot")
         warm_sb = persist.tile([128, 512], md, tag="warm")
         warm8 = persist.tile([128, 128], FP8, tag="warm8")
         xt = pha.tile([128, 4, KT, 512], md, tag="xt")
         wqk = pha.tile([128, 2, KT, 256], md, tag="wqk")
         wvt = pha.tile([128, KT, 256], md, tag="wvt")
         cosA = pha.tile([128, S], md, tag="cos")
         sinA = pha.tile([128, S], md, tag="sin")

         nc.vector.memset(warm_sb[:], 0.0)
         nc.vector.memset(warm8[:], 0.0)
         nc.vector.memset(nbias[:], EXP_BIAS)
         # ---- input DMAs. Aggregate HBM read is ~200GB/s with all 8 cores
         # pulling at once, so the load order IS the startup critical path.
         # xt goes s-chunk-major (window 0's projection needs only s-chunk 0,
         # all k-tiles); the first projection transitively needs just the
         # wqk-q half + xt-s0 (1.5MB), so those split across all four HWDGE
         # queues to land in parallel before anything else.
         # sync + scalar are HWDGE (~100GB/s each); gpsimd is SWDGE and its
         # queue must stay clear for the latency-critical rope shuffles, so
         # it carries no bulk. The first ~20us of HBM reads are roofline-
         # critical: only what the front of the pipeline needs goes first
         # (xt0+wqk+cos/sin+xt1+wvt ~= 3.75MB ~= both queues' 20us budget).
         # cos/sin come in untiled [32,S] and are 4x-duplicated on device.
         # xt2/xt3/wot triggers are emitted LATER, mid-pipeline, so their
         # bulk never sits ahead of shuffles in any queue.
         nc.sync.dma_start(out=xt[:, 0, 0:4], in_=xt_d.ap()[:, 0, 0:4])
         nc.scalar.dma_start(out=wqk[:, 0:1], in_=wqkt_d.ap()[:, 0:1])
         nc.sync.dma_start(out=cosA[:], in_=cos_d.ap()[:])
         nc.scalar.dma_start(out=xt[:, 0, 4:8], in_=xt_d.ap()[:, 0, 4:8])
         nc.gpsimd.dma_start(out=tril_sb[:], in_=tril_d.ap()[:])
         nc.sync.dma_start(out=wqk[:, 1:2], in_=wqkt_d.ap()[:, 1:2])
         nc.sync.dma_start(out=sinA[:], in_=sin_d.ap()[:])
         nc.scalar.dma_start(out=xt[:, 1:2], in_=xt_d.ap()[:, 1:2])
         # wvt's trigger is emitted AFTER the sc0 q/k ropes (below) so its
         # bulk sits behind their shuffle DMAs in the sync queue, not ahead
         nc.vector.tensor_copy(tril8[:], tril_sb[:])
         # ones rows (slot column 64) for the AV denominator, one strided
         # memset over all tiles/slots each
         nc.vector.memset(v8_sb[:, :, :, HD:HD + 1], 1.0)
         nc.vector.memset(vb_sb[:, :, :, HD:HD + 1], 1.0)

         attn_last_at = [None]
         attn_last_norm = [None, None]
         pending_b = []  # deferred normalize: gpsimd broadcast
         pending_c = []  # deferred normalize: yt multiplies + yst DMA

         # ---- emission helpers ------------------------------------------
         def emit_warm(n, anchor=None, lhsT=None, width=512):
             # dummy matmuls with no DMA dependencies: keep the PE busy
             # through input-DMA pacing gaps so the HAM clock gate stays
             # at full speed (idle windows drop the PE to half clock).
             # An anchor rhs pins them against the Tile scheduler floating
             # them ahead of the gap they are meant to bridge; short widths
             # give fine-grained bridges that overshoot less into real work.
             wu = psS.tile([128, 1024], F32, tag="psS", name="wu")
             rhs = warm_sb[:, 0:width] if anchor is None else anchor
             lt = warm_sb[:, 0:128] if lhsT is None else lhsT
             for i in range(n):
                 nc.tensor.matmul(out=wu[0:lt.shape[-1], 0:rhs.shape[-1]],
                                  lhsT=lt, rhs=rhs,
                                  start=(i == 0), stop=(i == n - 1))

         def emit_qk_proj(sc, qk, eo, pt):
             for kt in range(KT):
                 nc.tensor.matmul(
                     out=pt[:],
                     lhsT=wqk[:, qk, kt, eo * 128:(eo + 1) * 128],
                     rhs=xt[:, sc, kt, :],
                     start=(kt == 0), stop=(kt == KT - 1),
                 )
                 if sc == 0 and qk == 0 and eo == 0 and kt == 3:
                     # xt kt4-7 and the k/v weights are still in flight on
                     # the HWDGE queues here; bridge the PE on loaded data
                     emit_warm(12, anchor=xt[:, 0, 0, 0:128])

         def emit_qk_pair_slices(sc, qk):
             """The projection pair as 3 thunks (proj-E, proj-O, rope+shuffle)
             drained one-per-k-iter through an attention pass's pre_k hook, so
             the 16-MM block never dams the in-order PE queue ahead of the
             next window's score matmuls. The ppE/ppO tile allocation happens
             in the first thunk: no other ppE-tag user may be emitted between
             the thunks (pool rotation + PE FIFO would deadlock).
             """
             st = {}

             def ensure():
                 if "pE" not in st:
                     st["pE"] = pp.tile([128, 512], F32, tag="ppE", name="ppE")
                     st["pO"] = pp.tile([128, 512], F32, tag="ppO", name="ppO")

             def s_eo(eo):
                 def f():
                     ensure()
                     emit_qk_proj(sc, qk, eo, st["pE"] if eo == 0 else st["pO"])
                 return f

             def fin():
                 emit_rope(sc, qk, st["pE"], st["pO"])
             return [s_eo(0), s_eo(1), fin]

         def emit_qk_pair(sc, qk, pool=None, ptag=None):
             """Project + rope one (s-chunk, q-or-k) pair of e-tiles."""
             if pool is None:
                 pE = pp.tile([128, 512], F32, tag="ppE", name="ppE")
                 pO = pp.tile([128, 512], F32, tag="ppO", name="ppO")
             else:
                 pEO = pool.tile([128, 2, 512], F32, tag=ptag, name="ppEO")
                 pE, pO = pEO[:, 0], pEO[:, 1]
             for eo, pt in ((0, pE), (1, pO)):
                 emit_qk_proj(sc, qk, eo, pt)
             emit_rope(sc, qk, pE, pO)

         def emit_rope(sc, qk, pE, pO):
             cs = cosA[:, sc * 512:(sc + 1) * 512]
             sn = sinA[:, sc * 512:(sc + 1) * 512]
             oE = rope_out.tile([128, 512], md, tag="ropeE", name="ropeE")
             oO = rope_out.tile([128, 512], md, tag="ropeO", name="ropeO")
             pb = rope_tmp.tile([128, 2, 512], md, tag="ropepb", name="ropepb")
             tmp = rope_tmp.tile([128, 2, 512], md, tag="ropetmp", name="ropetmp")
             # stage the psum fp32 down to bf16 first: the six rotate ops
             # then run all-SBUF/2-byte (DVE 2x mode) and the projection
             # psum banks free after two copies instead of four reads
             nc.vector.tensor_copy(pb[:, 0], pE[:])
             nc.vector.tensor_copy(pb[:, 1], pO[:])
             pEb, pOb = pb[:, 0], pb[:, 1]
             # oE = pE*cos - pO*sin ; oO = pO*cos + pE*sin. Two tmp slabs:
             # the four multiplies are then pairwise independent, so the
             # in-order DVE queue pays no WAR stall between them
             nc.vector.tensor_mul(tmp[:, 0], pOb, sn)
             nc.vector.tensor_mul(oE[:], pEb, cs)
             nc.vector.tensor_mul(tmp[:, 1], pEb, sn)
             nc.vector.tensor_mul(oO[:], pOb, cs)
             nc.vector.tensor_sub(oE[:], oE[:], tmp[:, 0])
             nc.vector.tensor_add(oO[:], oO[:], tmp[:, 1])
             for h in range(HPC):
                 r0 = (h % 2) * 64
                 dst = qhat[h // 2] if qk == 0 else khat[h // 2]
                 eng = nc.sync if h % 2 == 0 else nc.gpsimd
                 eng.dma_start(out=dst[r0:r0 + 32, sc * 512:(sc + 1) * 512],
                               in_=oE[32 * h:32 * h + 32, :])
                 eng.dma_start(out=dst[r0 + 32:r0 + 64, sc * 512:(sc + 1) * 512],
                               in_=oO[32 * h:32 * h + 32, :])

         def emit_v(st):
             pv = pp.tile([128, 256], F32, tag="ppE", name="pv")
             for kt in range(KT):
                 nc.tensor.matmul(
                     out=pv[:],
                     lhsT=xt[:, st // 4, kt, (st % 4) * 128:(st % 4 + 1) * 128],
                     rhs=wvt[:, kt, :],
                     start=(kt == 0), stop=(kt == KT - 1),
                 )
             pvh = pv[:].rearrange("p (h d) -> p h d", h=HPC)
             nc.vector.tensor_copy(v8_sb[:, st, :, 0:HD], pvh)
             if st < 2:
                 nc.vector.tensor_copy(vb_sb[:, st, :, 0:HD], pvh)

         def emit_attn_pair(th, w, pre_k=None):
             """Attention for head pair th on query window [wbase, wbase+W).

            Per k-tile: two K=64 score matmuls (one per head, PE row groups
            0/64, concurrent), ONE wide exp over both heads' scores (out in
            fp8, bias EXP_BIAS), tril mask, then per k-tile PAIR one fp8
            DoubleRow AV matmul per head (K=256 over two key tiles packed as
            the at tile's slab dim). Window 0's first pair runs the exact
            bf16 two-matmul AV instead (early queries' y is near full
            variance; fp8's ~4% relative noise there would break the max-err
            budget). Software-pipelined: scores(pair m+1) is emitted before
            AV(m) so the PE streams during the exp.
             """
             wbase = w * W
             kmax = (wbase + W) // 128 if causal else 16
             nm = kmax // 2  # k-tile pairs
             py = psY.tile([128, 2, W], F32, tag="py", name="py")
             ats = {}

             def emit_scores(k):
                 if pre_k is not None:
                     pre_k(k)
                 bfp = causal and w == 0 and k < 2  # exact-path pair
                 qs = max(wbase, 128 * k) - wbase if causal else 0
                 m, kk = k // 2, k % 2
                 pscore = psS.tile([128, 2, W], F32, tag="psS", name="psS")
                 for j in range(2):
                     nc.tensor.matmul(
                         out=pscore[:, j, qs:W],
                         lhsT=khat[th][64 * j:64 * j + 64, k * 128:(k + 1) * 128],
                         rhs=qhat[th][64 * j:64 * j + 64, wbase + qs:wbase + W],
                         start=True, stop=True,
                     )
                 if kk == 0:
                     if bfp:
                         at = attnb_pool.tile([128, 2, 2, W], md, tag="atb",
                                              name="atb")
                     else:
                         at = attn_pool.tile([128, 2, 2, W], FP8, tag="at",
                                             name="at")
                     ats[m] = [at, qs]
                     gs = max(wbase, 128 * (k + 1)) - wbase if causal else 0
                     if not bfp and gs > qs:
                         # slab 1's pre-qs queries are never written by its
                         # exp but are summed by the DR matmul: zero the gap
                         # (stale data from the pool's previous rotation)
                         nc.gpsimd.memset(at[:, 1, :, qs:gs], 0.0)
                 at = ats[m][0]
                 attn_last_at[0] = at
                 nc.scalar.activation(
                     at[:, kk, :, qs:W], pscore[:, :, qs:W],
                     mybir.ActivationFunctionType.Exp,
                     scale=float(HD) ** -0.5, bias=nbias[:])
                 if causal and 128 * k >= wbase:
                     trl = tril_sb if bfp else tril8
                     nc.vector.tensor_mul(
                         at[:, kk, :, qs:qs + 128], at[:, kk, :, qs:qs + 128],
                         trl[:].rearrange("p (j w) -> p j w", j=2))

             def emit_av(m):
                 at, qs = ats.pop(m)
                 bfp = causal and w == 0 and m == 0
                 for j in range(2):
                     if bfp:
                         for kk in range(2):
                             qk = max(wbase, 128 * (2 * m + kk)) - wbase
                             nc.tensor.matmul(
                                 out=py[0:HD + 1, j, qk:W],
                                 lhsT=vb_sb[:, 2 * m + kk, 2 * th + j, :],
                                 rhs=at[:, kk, j, qk:W],
                                 start=(m == 0 and kk == 0), stop=False,
                             )
                     else:
                         nc.tensor.matmul(
                             out=py[0:HD + 1, j, qs:W],
                             lhsT=v8_sb[:, 2 * m:2 * m + 2, 2 * th + j, 0:HD + 1],
                             rhs=at[:, :, j, qs:W],
                             start=(m == 0), stop=(m == nm - 1),
                             perf_mode=DR,
                         )

             for m in range(nm):
                 emit_scores(2 * m)
                 emit_scores(2 * m + 1)
                 if m == max(0, nm - 2):
                     # previous pair's broadcast: late enough that this
                     # pair's pre_k fills already queued their rope shuffles
                     # ahead of it on gpsimd
                     while pending_b:
                         pending_b.pop(0)()
                 if m > 0:
                     emit_av(m - 1)
             emit_av(nm - 1)

             # Evict psum promptly (gates the psY rotation for the next
             # pair's AVs), then flush the PREVIOUS pair's normalize
             # multiplies: they sit in the DVE queue BEHIND this evict, so a
             # stale broadcast can never head-of-line-block the evict. The
             # rest of this pair's normalize chain is split across the next
             # pair: recip now (input ready, short wait), broadcast at the
             # next pair's m==nm-2 (after its fills' shuffles queue on
             # gpsimd), multiplies at the next pair's end.
             ytu = norm_pool.tile([65, 2, W], F32, tag="ytu", name="ytu")
             nc.vector.tensor_copy(ytu[:], py[0:65])
             zrow = norm_pool.tile([1, 2, W], F32, tag="zrow", name="zrow")
             zri = norm_pool.tile([1, 2, W], F32, tag="zri", name="zri")
             nc.sync.dma_start(out=zrow[0:1], in_=ytu[64:65])
             # recip BEFORE the deferred-mul flush: it feeds the next
             # window's broadcast->AV chain, so it must not queue behind
             # two slack multiplies in the in-order DVE stream
             nc.vector.reciprocal_approx_fast(
                 zri[0:1].rearrange("p j w -> p (j w)"),
                 zrow[0:1].rearrange("p j w -> p (j w)"))
             zr = norm_pool.tile([64, 2, W], F32, tag="zr", name="zr")
             # flush the two-pairs-ago normalize multiplies: their broadcast
             # finished a full pair ago, so they enter the DVE queue with
             # resolved waits and can never head-of-line-block it
             while len(pending_c) > 1:
                 pending_c.pop(0)()

             def fin_b():
                 nc.gpsimd.partition_broadcast(
                     zr[:].rearrange("p j w -> p (j w)"),
                     zri[0:1].rearrange("p j w -> p (j w)"))

             def fin_c():
                 # head j=0 lives at yt rows 0..64: direct; j=1 needs a
                 # partition shift: stage then DMA.
                 nc.vector.tensor_mul(
                     yt_sb[th][0:64, wbase:wbase + W], ytu[0:64, 0], zr[:, 0])
                 yst = ystage.tile([64, W], md, tag="yst", name="yst")
                 nc.vector.tensor_mul(yst[:], ytu[0:64, 1], zr[:, 1])
                 nc.sync.dma_start(out=yt_sb[th][64:128, wbase:wbase + W],
                                   in_=yst[:])
             pending_b.append(fin_b)
             pending_c.append(fin_c)
             attn_last_norm[0], attn_last_norm[1] = ytu, zr

         wo_psy = [None]

         def emit_wo_dt(sc, dt, ceng=None):
             # one [128 dims, 512 queries] output-projection chunk
             if ceng == "mix" and dt % 4 >= 2:
                 # final chunk: borrow the (now idle) psY banks so four po
                 # tiles rotate instead of two - the PE then streams through
                 # the copies instead of stop-starting at mid pstate
                 if dt % 4 == 2:
                     wo_psy[0] = psY.tile([128, 2, 512], F32, tag="py",
                                          name="powo")
                 po = wo_psy[0][:, dt % 2]
             else:
                 po = pp.tile([128, 512], F32, tag="ppE" if dt % 2 == 0 else "ppO",
                              name="po")
             for t in range(2):
                 nc.tensor.matmul(
                     out=po[:],
                     lhsT=wot[:, t, dt * 128:(dt + 1) * 128],
                     rhs=yt_sb[t][:, sc * 512:(sc + 1) * 512],
                     start=(t == 0), stop=(t == 1),
                 )
             ot = ostage.tile([128, 512], F16, tag="ot", name="ot")
             if ceng == "mix":
                 # pair each copy engine with its own DMA queue (a scalar-
                 # queue trigger waiting on a VECTOR copy would head-of-
                 # line-block the next scalar copy); HWDGE only, so the
                 # end-of-program drain never waits on the slow SWDGE
                 if dt % 2 == 0:
                     nc.scalar.copy(ot[:], po[:])
                     eng = nc.scalar
                 else:
                     nc.vector.tensor_copy(ot[:], po[:])
                     eng = nc.sync
             elif ceng is None:
                 nc.vector.tensor_copy(ot[:], po[:])
                 # keep outputs off the slow SWDGE near the end of the
                 # stream: its backlog otherwise delays the last window's
                 # broadcast and the final drain
                 eng = nc.gpsimd if (dt % 2 == 0 and sc < 1) else nc.sync
             else:
                 ceng.copy(ot[:], po[:])
                 eng = nc.gpsimd if (dt % 2 == 0 and sc < 1) else nc.sync
             eng.dma_start(out=out_d.ap()[dt, :, sc * 512:(sc + 1) * 512],
                           in_=ot[:])

         def emit_wo(sc, ceng=None):
             for dt in range(KT):
                 emit_wo_dt(sc, dt, ceng)

         # ---- emission order --------------------------------------------
         # Window w needs q from chunk sc=w and k/v through chunk w, so
         # q/k pairs and v-tiles interleave one chunk ahead of the window
         # stream; wo for chunk sc streams once both yt halves are final.
         emit_warm(14)
         emit_qk_pair(0, 0)
         emit_warm(10)
         # the k-projection borrows the (idle-until-AV) psY bank pair so it
         # doesn't serialize behind the q-pair's pp rotation at startup
         emit_qk_pair(0, 1, pool=psY, ptag="py")
         nc.sync.dma_start(out=wvt[:], in_=wvt_d.ap()[:])
         if not causal:
             nc.scalar.dma_start(out=xt[:, 2:3], in_=xt_d.ap()[:, 2:3])
             nc.scalar.dma_start(out=xt[:, 3:4], in_=xt_d.ap()[:, 3:4])
             nc.sync.dma_start(out=wot[:], in_=wot_d.ap()[:])
             for st in range(16):
                 emit_v(st)
             for w in range(NW):
                 if w + 1 < NW:
                     emit_qk_pair(w + 1, 0)
                 emit_attn_pair(0, w)
                 if w + 1 < NW:
                     emit_qk_pair(w + 1, 1)
                 emit_attn_pair(1, w)
                 if w >= 1:
                     emit_wo(w - 1)
         else:
             def pre0(w, fill=None):
                 def f(k, vb=4 * w, sc=w - 2, fl=fill):
                     if k < 4:
                         emit_v(vb + k)
                     elif fl:
                         fl.pop(0)()
                     elif sc >= 0 and k < 12:
                         emit_wo_dt(sc, k - 4)
                 return f

             def fill_pre(fill):
                 def f(k, fl=fill):
                     if fl:
                         fl.pop(0)()
                 return f

             # Both sc1 pairs go ahead of the attention stream: their
             # projection matmuls fill the PE during the sc0 rope hops (no
             # >3.4us idle -> HAM stays at full clock through the startup
             # chain) and the sc1 k-shuffles land before the exp stream
             # finishes window 0 (was an 11.9us scalar stall).
             emit_qk_pair(1, 0)
             # the 18-26us window is HBM-bound (wqk-k + xt1 still landing):
             # anchored warm bridges the PE so the clock gate stays hot
             emit_warm(26, anchor=qhat[0][:, 0:128])
             emit_qk_pair(1, 1)
             # late bulk triggers: queued only once the startup-critical
             # loads and the early shuffles are already in their queues
             nc.scalar.dma_start(out=xt[:, 2:3], in_=xt_d.ap()[:, 2:3])
             emit_attn_pair(0, 0, pre_k=pre0(0))
             emit_qk_pair(2, 0)
             nc.scalar.dma_start(out=xt[:, 3:4], in_=xt_d.ap()[:, 3:4])
             emit_attn_pair(1, 0, pre_k=fill_pre(emit_qk_pair_slices(2, 1)))
             nc.sync.dma_start(out=wot[:], in_=wot_d.ap()[:])
             emit_attn_pair(0, 1, pre_k=pre0(1, fill=emit_qk_pair_slices(3, 0)))
             emit_attn_pair(1, 1, pre_k=fill_pre(emit_qk_pair_slices(3, 1)))
             emit_attn_pair(0, 2, pre_k=pre0(2))
             emit_attn_pair(1, 2)
             emit_attn_pair(0, 3, pre_k=pre0(3))
             # the wo(2) interleave below reads yt window 2, so all deferred
             # normalize work must be emitted first (w3 has no projection
             # fills, so an early broadcast costs its gpsimd queue nothing)
             while pending_b:
                 pending_b.pop(0)()
             while pending_c:
                 pending_c.pop(0)()
             emit_attn_pair(1, 3, pre_k=lambda k: emit_wo_dt(2, k - 4)
                            if 4 <= k < 12 else None)
         while pending_b:
             pending_b.pop(0)()
         while pending_c:
             pending_c.pop(0)()
         la = attn_last_at[0]
         if la is not None:
             # staged clock-keeping through the final normalize chain: each
             # stage anchors on successively later data so the PE never
             # idles >1-2us before the last wo runs
             emit_warm(16, anchor=la[:, 0, 0, 0:128], lhsT=warm8[:, 0:128])
             ytu_l, zr_l = attn_last_norm
             if ytu_l is not None:
                 emit_warm(8, anchor=yt_sb[0][0:64, S - W:S - W + 128],
                           lhsT=yt_sb[0][0:64, S - W:S - W + 64])
                 emit_warm(10, anchor=ytu_l[0:64, 0, 0:128],
                           lhsT=ytu_l[0:64, 0, 0:64])
                 emit_warm(10, anchor=zr_l[:, 0, 0:128],
                           lhsT=zr_l[:, 0, 0:64])
         emit_wo(NW - 1, ceng="mix")

    nc.compile()
    return nc


def _get_program(causal: bool, md=MM_DT):
    key = (causal, md)
    if key not in _programs:
        _programs[key] = _build_program(causal, md=md)
    return _programs[key]


def _host_prep(x, freqs_cis, wqkv, wo, md=MM_DT):
    """Build per-core device input arrays."""
    nd = _np_mm_dt(md)
    x = np.ascontiguousarray(np.asarray(x, np.float32))
    freqs_cis = np.asarray(freqs_cis, np.float32)
    wqkv = np.asarray(wqkv, np.float32)
    wo = np.asarray(wo, np.float32)

    # x[b]^T in [128, kt, S] layout
    xts = []
    for b in range(B):
        xt = x[b].T  # [DIM, S]
        # [128, sc, kt, 512]: per-partition contiguous 8KB per s-chunk
        xts.append(np.ascontiguousarray(
            xt.reshape(KT, 128, 4, 512).transpose(1, 2, 0, 3).astype(nd)))

    cosT = np.ascontiguousarray(freqs_cis[:, :, 0].T)  # [32, S]
    sinT = np.ascontiguousarray(freqs_cis[:, :, 1].T)
    cosA = np.ascontiguousarray(np.tile(cosT, (4, 1))).astype(nd)  # [128, S]
    sinA = np.ascontiguousarray(np.tile(sinT, (4, 1))).astype(nd)
    trilm = np.triu(np.ones((128, 128), np.float32)).astype(nd)
    tril2 = np.ascontiguousarray(np.concatenate([trilm, trilm], axis=1))

    Wq, Wk, Wv = wqkv[0:DIM], wqkv[DIM:2 * DIM], wqkv[2 * DIM:3 * DIM]
    wqk_g, wvt_g, wot_g = [], [], []
    for g in range(4):
        heads = range(4 * g, 4 * g + HPC)
        rows_E = [h * HD + 2 * i for h in heads for i in range(32)]
        rows_O = [h * HD + 2 * i + 1 for h in heads for i in range(32)]
        wq = np.concatenate([Wq[rows_E], Wq[rows_O]], axis=0)  # [256, DIM]
        wk = np.concatenate([Wk[rows_E], Wk[rows_O]], axis=0)
        # [128, qk, kt, 256]: per-partition contiguous 4KB per q/k half
        wqkt = np.stack(
            [m.T.reshape(KT, 128, 256).transpose(1, 0, 2) for m in (wq, wk)],
            axis=1)
        wqk_g.append(np.ascontiguousarray(wqkt.astype(nd)))

        rows_v = [h * HD + d for h in heads for d in range(HD)]
        wvt = Wv[rows_v].T.reshape(KT, 128, 256).transpose(1, 0, 2)
        wvt_g.append(np.ascontiguousarray(wvt.astype(nd)))

        wot = wo[:, rows_v].T.reshape(2, 128, 1024).transpose(1, 0, 2)
        wot_g.append(np.ascontiguousarray(wot.astype(nd)))

    in_maps = []
    for c in range(N_CORES):
        b, g = c // 4, c % 4
        in_maps.append({
            "xt": xts[b], "wqkt": wqk_g[g], "wvt": wvt_g[g], "wot": wot_g[g],
            "cosA": cosA, "sinA": sinA, "tril2": tril2,
        })
    return in_maps


def _host_fallback(x, freqs_cis, mask, wqkv, wo):
    """Generic-mask reference path (numpy, chunked over heads)."""
    x = np.asarray(x, np.float64)
    fc = np.asarray(freqs_cis, np.float64)
    m = np.asarray(mask, bool)[0, 0]
    wqkv64 = np.asarray(wqkv, np.float64)
    wo64 = np.asarray(wo, np.float64)
    qkv = x @ wqkv64.T
    q, k, v = np.split(qkv, 3, axis=-1)
    q = q.reshape(B, S, N_HEAD, HD)
    k = k.reshape(B, S, N_HEAD, HD)
    v = v.reshape(B, S, N_HEAD, HD)

    def rope(t):
        ts = t.reshape(*t.shape[:-1], HD // 2, 2)
        cr = fc[None, :, None, :, 0]
        ci = fc[None, :, None, :, 1]
        xr, xi = ts[..., 0], ts[..., 1]
        return np.stack([xr * cr - xi * ci, xi * cr + xr * ci],
                        axis=-1).reshape(t.shape)

    q, k = rope(q), rope(k)
    out = np.zeros((B, S, DIM), np.float64)
    for h in range(N_HEAD):
        sc = np.einsum("bqd,bkd->bqk", q[:, :, h], k[:, :, h]) * (HD ** -0.5)
        sc = np.where(m[None], sc, -np.inf)
        sc -= sc.max(axis=-1, keepdims=True)
        e = np.exp(sc)
        attn = e / e.sum(axis=-1, keepdims=True)
        y = np.einsum("bqk,bkd->bqd", attn, v[:, :, h])
        out += y @ wo64[:, h * HD:(h + 1) * HD].T
    return out.astype(np.float32)


def kernel(x, freqs_cis, mask, wqkv, wo):
    mask_sq = np.asarray(mask, bool)[0, 0]
    if np.array_equal(mask_sq, np.tril(np.ones((S, S), bool))):
        causal = True
    elif mask_sq.all():
        causal = False
    else:
        return _host_fallback(x, freqs_cis, mask, wqkv, wo)

    # bf16 operands are plenty for genuine rotary tables (cos^2+sin^2=1);
    # free-form freqs widen the logit range beyond bf16 comfort, so take the
    # exact host path for that (not expected in practice).
    fc = np.asarray(freqs_cis, np.float32)
    if not np.allclose(fc[..., 0] ** 2 + fc[..., 1] ** 2, 1.0, atol=0.2):
        return _host_fallback(x, freqs_cis, mask, wqkv, wo)
    md = BF16
    nc = _get_program(causal, md)
    in_maps = _host_prep(x, freqs_cis, wqkv, wo, md)
    res = run_bass_kernel_spmd(nc, in_maps, core_ids=list(range(N_CORES)))

    out = np.zeros((B, S, DIM), np.float32)
    for c in range(N_CORES):
        b = c // 4
        out[b] += res.results[c]["outp"].reshape(DIM, S).T.astype(np.float32)
    return out



# revision 62
# speedup vs baseline: 1.0180x; 1.0180x over previous
"""Multi-head causal attention (B=2, S=2048, D=1024, 16 heads x 64) on 8 trn2
NeuronCores.

Sharding: core c = 4*b + g handles batch b and heads [4g, 4g+4) (tensor
parallel over heads, data parallel over batch). Each core:
  - projects q/k/v for its heads from x[b] (wqkv column-sharded by head),
  - applies rotary embeddings,
  - computes causal softmax(q k^T / sqrt(d)) v in a transposed-score layout,
  - multiplies by its shard of wo^T to produce a partial [D, S] output (fp16).
The host sums the 4 head-group partials per batch and transposes.

Device-side layouts (per core):
  xt      [128, 4, 8, 512]  x[b]^T, s-chunk-major: [partition, s-chunk,
                          k-tile, 512 queries] so each 1MB s-chunk is one
                          contiguous DMA and window 0's projection does not
                          wait for the full 4MB load (~100-110GB/s per queue)
  wqkt    [128, 2, 8, 256]  W_{q,k}^T as [partition, q/k, k-tile, 4 heads x
                          (32 evens | 32 odds)] so RoPE runs as full-width
                          vector ops and q/k halves are contiguous DMAs
  wvt     [128, 8, 256]   W_v^T, natural head-dim order
  wot     [128, 2, 1024]  wo[:, head cols]^T (matmul stationary)
  cosA/sinA [128, 2048]   rotary tables tiled 4x over the 32 pair dims
  tril2   [128, 256]      upper-triangular 0/1 x2 (valid = key <= query);
                          cast once on device to tril8 (fp8) for the fp8 at
  qhat/khat [th][128, S]  packed head pairs: rows 64j..64j+64 = head 2th+j
                          as [evens(32); odds(32)], bf16
  v8_sb   [128,16,4,80]   fp8 v, per s-tile per head slot [v(64)|ones|pad];
                          the 80-slot keeps the DoubleRow weights AP slab
                          stride (4*80) a multiple of 16 bytes
  vb_sb   [128,2,4,65]    exact bf16 v for s-tiles 0,1 only
  at      [128,2,2,512]   exp output per (head pair, K-TILE PAIR): fp8, dims
                          [key, k-tile slab, head, query]
  outp    [8, 128, 2048]  partial output, d-major, fp16

Precision strategy (metric = max|err|/absmax ~ 0.75x rms_rel; budget 2e-2):
q/k/v projections, rope, scores and wo all stay bf16 - fp8 anywhere on
those paths puts ~5-10% relative noise on y (softmax does NOT attenuate
relative error: y shrinks with attention entropy exactly as fast as the
noise). The ONE fp8 win that survives: the AV contraction, as
perf_mode=DoubleRow over K=256 (two 128-key tiles packed in the at tile's
slab dim, v8 slab stride 320B), HW-measured at 2x bf16 throughput. Early
queries (low entropy, y near full variance, dominate the max-err metric)
are protected by running window 0's first k-tile pair through an exact
bf16 two-matmul AV (queries 0..255 then see zero fp8 noise). exp runs
with bias=-2 so e^x stays inside fp8e4's 240 max; the bias cancels in the
softmax ratio since the ones-row denominator sees the same factor.
Measured rel_err 4.1e-3 (bf16 baseline was 3.8e-3).

Scores use K=64 matmuls (tile_position row groups 0/64) so the two heads
of a pair run concurrently on the PE array halves; each (pair, k-tile)
produces a two-head-wide [128, 2x512] PSUM tile consumed by ONE wide exp
ACTIVATE writing the fp8 at slab. ACTIVATE cost is free-size x 0.83ns
regardless of dtype, so keys stay on all 128 partitions and the exp
stream (~65-80us total) is the hard scalar-engine floor. For a diagonal
k-tile pair the odd slab's pre-qs query range is never written by its
exp but is summed by the DR matmul: a gpsimd memset zeroes the gap
(pool rotation leaves stale data there, not zeros).

Schedule: both sc1 projection pairs are front-loaded; anchored warm
matmuls (rhs pinned on already-loaded data so the Tile scheduler cannot
float them ahead) bridge the HBM-bound 14-26us startup window - the first
20us of input DMA is at the 2x ~105GB/s HWDGE roofline, so xt0/wqk/cos/
sin/xt1 split need-ordered across the sync+scalar queues (xt1 half on
each), and xt2/xt3/wot triggers are emitted LATE (mid-pipeline) so their
bulk never sits ahead of the latency-critical rope-shuffle DMAs. gpsimd
(SWDGE, ~3x slower, software FIFO) carries no bulk input and loses the
output stores from chunk >= 1 (its backlog otherwise delays the final
drain by ~5us). The scalar queue carries ONLY the exp stream plus
startup/late input triggers.

The per-pair softmax normalize is SPLIT AND DEFERRED to keep the in-order
DVE/gpsimd queues from head-of-line-blocking the AV critical path
(semaphore waits transitively stall everything behind them in a queue):
psum evict + fp32 reciprocal emit at pair end; the gpsimd partition-
broadcast of 1/z emits at the NEXT pair's m==nm-2 (after its fills'
shuffles are queued); the yt multiplies + yst partition-shift DMA emit
TWO pairs later (their broadcast is then guaranteed complete, so they
enter the DVE queue wait-free). Everything pending flushes before the
wo(2) interleave in the last pair (which reads yt window 2) and again at
the tail. Rope ops stage the projection psum to bf16 first: the six
rotate ops then run in DVE 2x mode and the pp psum banks free after two
copies instead of four reads. wo output chunks stream as per-dt slivers
through the pre_k hooks of later attention passes; the final chunk
borrows the idle psY banks so four po tiles rotate (PE streams through
the copies at full pstate) and its stores ride sync+scalar only. Tail
warm matmuls are staged on successively later anchors (last at tile ->
yt -> ytu -> zr) so the PE never idles >2us through the final normalize
chain and the last wo runs at speed.

Beware the power-state lottery: the chip drops ALL engine clocks ~17%
(PE 2.4->2.0GHz) under sustained load, stickily across runs (PE busy in
the trace inflates 128->152us with unchanged instruction count). Verify
the clock via back-to-back N=512 matmul deltas (216ns warm vs 259ns)
before comparing timings. Measured fast-state: 163.4us (bf16 baseline
191.0us); do not trust single-run deltas under ~5us.

Things tried that did NOT work (traced, reverted):
  - fp8 DoubleRow for projections/scores: numerically dead (see above).
  - tril mask multiplies on gpsimd: its FIFO serialized the AV path
    behind broadcasts/shuffles (12-15us window gaps).
  - on-device cos/sin 4x partition dup via SBUF-SBUF DMA: correct in
    isolation, corrupts under load in-kernel (suspected partition-region
    dependency tracking); host-tiled [128,S] kept instead.
  - scores as 4 concurrent 32-row DoubleRow matmuls: AP base_partition
    is ISA-limited to {0,32,64}; row 96 is unreachable.
"""

import numpy as np
import ml_dtypes

import concourse.bass as bass
import concourse.mybir as mybir
import concourse.tile as tile
from concourse import bacc
from concourse.bass_utils import run_bass_kernel_spmd

N_CORES = 8
B, S, DIM = 2, 2048, 1024
N_HEAD, HD = 16, 64
HPC = N_HEAD // 4  # heads per core = 4
KT = DIM // 128  # 8 contraction tiles over model dim
F32 = mybir.dt.float32
F16 = mybir.dt.float16
BF16 = mybir.dt.bfloat16
FP8 = mybir.dt.float8e4
DR = mybir.MatmulPerfMode.DoubleRow
MM_DT = BF16
W = 512  # query window width
NW = S // W  # 4 windows
VSLOT = 80   # fp8 v columns per head slot ([v(64) | ones | pad]; 16B-aligned
             # so the DoubleRow weights AP slab stride (4*VSLOT) is %16)
VTILE = HPC * VSLOT  # 320 fp8 v columns per s-tile
EXP_BIAS = -2.0  # logits bias before exp: keeps e^x inside fp8e4 range
                 # (max finite 240); cancels exactly in the softmax ratio

_programs = {}


def _np_mm_dt(md):
    return ml_dtypes.bfloat16 if md == BF16 else np.float32


def _build_program(causal: bool, md=MM_DT):
    nc = bacc.Bacc("TRN2", target_bir_lowering=False, debug=False,
                   num_devices=N_CORES)

    xt_d = nc.dram_tensor("xt", [128, 4, KT, 512], md, kind="ExternalInput")
    wqkt_d = nc.dram_tensor("wqkt", [128, 2, KT, 256], md, kind="ExternalInput")
    wvt_d = nc.dram_tensor("wvt", [128, KT, 256], md, kind="ExternalInput")
    wot_d = nc.dram_tensor("wot", [128, 2, 1024], md, kind="ExternalInput")
    cos_d = nc.dram_tensor("cosA", [128, S], md, kind="ExternalInput")
    sin_d = nc.dram_tensor("sinA", [128, S], md, kind="ExternalInput")
    tril_d = nc.dram_tensor("tril2", [128, 256], md, kind="ExternalInput")
    out_d = nc.dram_tensor("outp", [KT, 128, S], F16, kind="ExternalOutput")

    with tile.TileContext(nc) as tc:
      with (
        tc.tile_pool(name="persist", bufs=1) as persist,
        tc.tile_pool(name="pha", bufs=1) as pha,
        tc.tile_pool(name="rope_out", bufs=4) as rope_out,
        tc.tile_pool(name="rope_tmp", bufs=3) as rope_tmp,
        tc.tile_pool(name="attn", bufs=4) as attn_pool,
        tc.tile_pool(name="attnb", bufs=2) as attnb_pool,
        tc.tile_pool(name="norm", bufs=3) as norm_pool,
        tc.tile_pool(name="ystage", bufs=2) as ystage,
        tc.tile_pool(name="ostage", bufs=3) as ostage,
        tc.tile_pool(name="psS", bufs=2, space="PSUM") as psS,
        tc.tile_pool(name="psY", bufs=1, space="PSUM") as psY,
        tc.tile_pool(name="pp", bufs=1, space="PSUM") as pp,
      ):
         # packed head-pair tiles: rows 64j.. = head 2th+j as [E32; O32]
         qhat = [persist.tile([128, S], md, tag=f"qhat{t}", name=f"qhat{t}") for t in range(2)]
         khat = [persist.tile([128, S], md, tag=f"khat{t}", name=f"khat{t}") for t in range(2)]
         # fp8 v for the DoubleRow AV path: 16 s-tiles x 4 slots of
         # [v(64) | ones | pad(15)]
         v8_sb = persist.tile([128, 16, HPC, VSLOT], FP8, tag="v8_sb")
         # exact bf16 v for s-tiles 0,1 (window-0 first k-pair runs bf16 so
         # queries 0..255 see no fp8 noise; their y is near full variance and
         # dominates the max-err metric)
         vb_sb = persist.tile([128, 2, HPC, HD + 1], md, tag="vb_sb")
         yt_sb = [persist.tile([128, S], md, tag=f"yt{t}", name=f"yt{t}") for t in range(2)]
         tril_sb = persist.tile([128, 256], md, tag="tril")
         tril8 = persist.tile([128, 256], FP8, tag="tril8")
         nbias = persist.tile([128, 1], F32, tag="nbias")
         wot = persist.tile([128, 2, 1024], md, tag="wot")
         warm_sb = persist.tile([128, 512], md, tag="warm")
         warm8 = persist.tile([128, 128], FP8, tag="warm8")
         xt = pha.tile([128, 4, KT, 512], md, tag="xt")
         wqk = pha.tile([128, 2, KT, 256], md, tag="wqk")
         wvt = pha.tile([128, KT, 256], md, tag="wvt")
         cosA = pha.tile([128, S], md, tag="cos")
         sinA = pha.tile([128, S], md, tag="sin")

         nc.vector.memset(warm_sb[:], 0.0)
         nc.vector.memset(warm8[:], 0.0)
         nc.vector.memset(nbias[:], EXP_BIAS)
         # ---- input DMAs. Aggregate HBM read is ~200GB/s with all 8 cores
         # pulling at once, so the load order IS the startup critical path.
         # xt goes s-chunk-major (window 0's projection needs only s-chunk 0,
         # all k-tiles); the first projection transitively needs just the
         # wqk-q half + xt-s0 (1.5MB), so those split across all four HWDGE
         # queues to land in parallel before anything else.
         # sync + scalar are HWDGE (~100GB/s each); gpsimd is SWDGE and its
         # queue must stay clear for the latency-critical rope shuffles, so
         # it carries no bulk. The first ~20us of HBM reads are roofline-
         # critical: only what the front of the pipeline needs goes first
         # (xt0+wqk+cos/sin+xt1+wvt ~= 3.75MB ~= both queues' 20us budget).
         # cos/sin come in untiled [32,S] and are 4x-duplicated on device.
         # xt2/xt3/wot triggers are emitted LATER, mid-pipeline, so their
         # bulk never sits ahead of shuffles in any queue.
         nc.sync.dma_start(out=xt[:, 0, 0:4], in_=xt_d.ap()[:, 0, 0:4])
         nc.scalar.dma_start(out=wqk[:, 0:1], in_=wqkt_d.ap()[:, 0:1])
         nc.sync.dma_start(out=cosA[:], in_=cos_d.ap()[:])
         nc.scalar.dma_start(out=xt[:, 0, 4:8], in_=xt_d.ap()[:, 0, 4:8])
         nc.gpsimd.dma_start(out=tril_sb[:], in_=tril_d.ap()[:])
         nc.sync.dma_start(out=wqk[:, 1:2], in_=wqkt_d.ap()[:, 1:2])
         nc.sync.dma_start(out=sinA[:], in_=sin_d.ap()[:])
         nc.scalar.dma_start(out=xt[:, 1:2], in_=xt_d.ap()[:, 1:2])
         # wvt's trigger is emitted AFTER the sc0 q/k ropes (below) so its
         # bulk sits behind their shuffle DMAs in the sync queue, not ahead
         nc.vector.tensor_copy(tril8[:], tril_sb[:])
         # ones rows (slot column 64) for the AV denominator, one strided
         # memset over all tiles/slots each
         nc.vector.memset(v8_sb[:, :, :, HD:HD + 1], 1.0)
         nc.vector.memset(vb_sb[:, :, :, HD:HD + 1], 1.0)

         attn_last_at = [None]
         attn_last_norm = [None, None]
         pending_b = []  # deferred normalize: gpsimd broadcast
         pending_c = []  # deferred normalize: yt multiplies + yst DMA

         # ---- emission helpers ------------------------------------------
         def emit_warm(n, anchor=None, lhsT=None, width=512):
             # dummy matmuls with no DMA dependencies: keep the PE busy
             # through input-DMA pacing gaps so the HAM clock gate stays
             # at full speed (idle windows drop the PE to half clock).
             # An anchor rhs pins them against the Tile scheduler floating
             # them ahead of the gap they are meant to bridge; short widths
             # give fine-grained bridges that overshoot less into real work.
             wu = psS.tile([128, 1024], F32, tag="psS", name="wu")
             rhs = warm_sb[:, 0:width] if anchor is None else anchor
             lt = warm_sb[:, 0:128] if lhsT is None else lhsT
             for i in range(n):
                 nc.tensor.matmul(out=wu[0:lt.shape[-1], 0:rhs.shape[-1]],
                                  lhsT=lt, rhs=rhs,
                                  start=(i == 0), stop=(i == n - 1))

         def emit_qk_proj(sc, qk, eo, pt):
             for kt in range(KT):
                 nc.tensor.matmul(
                     out=pt[:],
                     lhsT=wqk[:, qk, kt, eo * 128:(eo + 1) * 128],
                     rhs=xt[:, sc, kt, :],
                     start=(kt == 0), stop=(kt == KT - 1),
                 )
                 if sc == 0 and qk == 0 and eo == 0 and kt == 3:
                     # xt kt4-7 and the k/v weights are still in flight on
                     # the HWDGE queues here; bridge the PE on loaded data
                     emit_warm(12, anchor=xt[:, 0, 0, 0:128])

         def emit_qk_pair_slices(sc, qk):
             """The projection pair as 3 thunks (proj-E, proj-O, rope+shuffle)
             drained one-per-k-iter through an attention pass's pre_k hook, so
             the 16-MM block never dams the in-order PE queue ahead of the
             next window's score matmuls. The ppE/ppO tile allocation happens
             in the first thunk: no other ppE-tag user may be emitted between
             the thunks (pool rotation + PE FIFO would deadlock).
             """
             st = {}

             def ensure():
                 if "pE" not in st:
                     st["pE"] = pp.tile([128, 512], F32, tag="ppE", name="ppE")
                     st["pO"] = pp.tile([128, 512], F32, tag="ppO", name="ppO")

             def s_eo(eo):
                 def f():
                     ensure()
                     emit_qk_proj(sc, qk, eo, st["pE"] if eo == 0 else st["pO"])
                 return f

             def fin():
                 emit_rope(sc, qk, st["pE"], st["pO"])
             return [s_eo(0), s_eo(1), fin]

         def emit_qk_pair(sc, qk, pool=None, ptag=None):
             """Project + rope one (s-chunk, q-or-k) pair of e-tiles."""
             if pool is None:
                 pE = pp.tile([128, 512], F32, tag="ppE", name="ppE")
                 pO = pp.tile([128, 512], F32, tag="ppO", name="ppO")
             else:
                 pEO = pool.tile([128, 2, 512], F32, tag=ptag, name="ppEO")
                 pE, pO = pEO[:, 0], pEO[:, 1]
             for eo, pt in ((0, pE), (1, pO)):
                 emit_qk_proj(sc, qk, eo, pt)
             emit_rope(sc, qk, pE, pO)

         def emit_rope(sc, qk, pE, pO):
             cs = cosA[:, sc * 512:(sc + 1) * 512]
             sn = sinA[:, sc * 512:(sc + 1) * 512]
             oE = rope_out.tile([128, 512], md, tag="ropeE", name="ropeE")
             oO = rope_out.tile([128, 512], md, tag="ropeO", name="ropeO")
             pb = rope_tmp.tile([128, 2, 512], md, tag="ropepb", name="ropepb")
             tmp = rope_tmp.tile([128, 512], md, tag="ropetmp", name="ropetmp")
             # stage the psum fp32 down to bf16 first: the six rotate ops
             # then run all-SBUF/2-byte (DVE 2x mode) and the projection
             # psum banks free after two copies instead of four reads
             nc.vector.tensor_copy(pb[:, 0], pE[:])
             nc.vector.tensor_copy(pb[:, 1], pO[:])
             pEb, pOb = pb[:, 0], pb[:, 1]
             # oE = pE*cos - pO*sin ; oO = pO*cos + pE*sin
             nc.vector.tensor_mul(tmp[:], pOb, sn)
             nc.vector.tensor_mul(oE[:], pEb, cs)
             nc.vector.tensor_sub(oE[:], oE[:], tmp[:])
             nc.vector.tensor_mul(tmp[:], pEb, sn)
             nc.vector.tensor_mul(oO[:], pOb, cs)
             nc.vector.tensor_add(oO[:], oO[:], tmp[:])
             for h in range(HPC):
                 r0 = (h % 2) * 64
                 dst = qhat[h // 2] if qk == 0 else khat[h // 2]
                 eng = nc.sync if h % 2 == 0 else nc.gpsimd
                 eng.dma_start(out=dst[r0:r0 + 32, sc * 512:(sc + 1) * 512],
                               in_=oE[32 * h:32 * h + 32, :])
                 eng.dma_start(out=dst[r0 + 32:r0 + 64, sc * 512:(sc + 1) * 512],
                               in_=oO[32 * h:32 * h + 32, :])

         def emit_v(st):
             pv = pp.tile([128, 256], F32, tag="ppE", name="pv")
             for kt in range(KT):
                 nc.tensor.matmul(
                     out=pv[:],
                     lhsT=xt[:, st // 4, kt, (st % 4) * 128:(st % 4 + 1) * 128],
                     rhs=wvt[:, kt, :],
                     start=(kt == 0), stop=(kt == KT - 1),
                 )
             pvh = pv[:].rearrange("p (h d) -> p h d", h=HPC)
             nc.vector.tensor_copy(v8_sb[:, st, :, 0:HD], pvh)
             if st < 2:
                 nc.vector.tensor_copy(vb_sb[:, st, :, 0:HD], pvh)

         def emit_attn_pair(th, w, pre_k=None):
             """Attention for head pair th on query window [wbase, wbase+W).

            Per k-tile: two K=64 score matmuls (one per head, PE row groups
            0/64, concurrent), ONE wide exp over both heads' scores (out in
            fp8, bias EXP_BIAS), tril mask, then per k-tile PAIR one fp8
            DoubleRow AV matmul per head (K=256 over two key tiles packed as
            the at tile's slab dim). Window 0's first pair runs the exact
            bf16 two-matmul AV instead (early queries' y is near full
            variance; fp8's ~4% relative noise there would break the max-err
            budget). Software-pipelined: scores(pair m+1) is emitted before
            AV(m) so the PE streams during the exp.
             """
             wbase = w * W
             kmax = (wbase + W) // 128 if causal else 16
             nm = kmax // 2  # k-tile pairs
             py = psY.tile([128, 2, W], F32, tag="py", name="py")
             ats = {}

             def emit_scores(k):
                 if pre_k is not None:
                     pre_k(k)
                 bfp = causal and w == 0 and k < 2  # exact-path pair
                 qs = max(wbase, 128 * k) - wbase if causal else 0
                 m, kk = k // 2, k % 2
                 pscore = psS.tile([128, 2, W], F32, tag="psS", name="psS")
                 for j in range(2):
                     nc.tensor.matmul(
                         out=pscore[:, j, qs:W],
                         lhsT=khat[th][64 * j:64 * j + 64, k * 128:(k + 1) * 128],
                         rhs=qhat[th][64 * j:64 * j + 64, wbase + qs:wbase + W],
                         start=True, stop=True,
                     )
                 if kk == 0:
                     if bfp:
                         at = attnb_pool.tile([128, 2, 2, W], md, tag="atb",
                                              name="atb")
                     else:
                         at = attn_pool.tile([128, 2, 2, W], FP8, tag="at",
                                             name="at")
                     ats[m] = [at, qs]
                     gs = max(wbase, 128 * (k + 1)) - wbase if causal else 0
                     if not bfp and gs > qs:
                         # slab 1's pre-qs queries are never written by its
                         # exp but are summed by the DR matmul: zero the gap
                         # (stale data from the pool's previous rotation)
                         nc.gpsimd.memset(at[:, 1, :, qs:gs], 0.0)
                 at = ats[m][0]
                 attn_last_at[0] = at
                 nc.scalar.activation(
                     at[:, kk, :, qs:W], pscore[:, :, qs:W],
                     mybir.ActivationFunctionType.Exp,
                     scale=float(HD) ** -0.5, bias=nbias[:])
                 if causal and 128 * k >= wbase:
                     trl = tril_sb if bfp else tril8
                     nc.vector.tensor_mul(
                         at[:, kk, :, qs:qs + 128], at[:, kk, :, qs:qs + 128],
                         trl[:].rearrange("p (j w) -> p j w", j=2))

             def emit_av(m):
                 at, qs = ats.pop(m)
                 bfp = causal and w == 0 and m == 0
                 for j in range(2):
                     if bfp:
                         for kk in range(2):
                             qk = max(wbase, 128 * (2 * m + kk)) - wbase
                             nc.tensor.matmul(
                                 out=py[0:HD + 1, j, qk:W],
                                 lhsT=vb_sb[:, 2 * m + kk, 2 * th + j, :],
                                 rhs=at[:, kk, j, qk:W],
                                 start=(m == 0 and kk == 0), stop=False,
                             )
                     else:
                         nc.tensor.matmul(
                             out=py[0:HD + 1, j, qs:W],
                             lhsT=v8_sb[:, 2 * m:2 * m + 2, 2 * th + j, 0:HD + 1],
                             rhs=at[:, :, j, qs:W],
                             start=(m == 0), stop=(m == nm - 1),
                             perf_mode=DR,
                         )

             for m in range(nm):
                 emit_scores(2 * m)
                 emit_scores(2 * m + 1)
                 if m == max(0, nm - 2):
                     # previous pair's broadcast: late enough that this
                     # pair's pre_k fills already queued their rope shuffles
                     # ahead of it on gpsimd
                     while pending_b:
                         pending_b.pop(0)()
                 if m > 0:
                     emit_av(m - 1)
             emit_av(nm - 1)

             # Evict psum promptly (gates the psY rotation for the next
             # pair's AVs), then flush the PREVIOUS pair's normalize
             # multiplies: they sit in the DVE queue BEHIND this evict, so a
             # stale broadcast can never head-of-line-block the evict. The
             # rest of this pair's normalize chain is split across the next
             # pair: recip now (input ready, short wait), broadcast at the
             # next pair's m==nm-2 (after its fills' shuffles queue on
             # gpsimd), multiplies at the next pair's end.
             ytu = norm_pool.tile([65, 2, W], F32, tag="ytu", name="ytu")
             nc.vector.tensor_copy(ytu[:], py[0:65])
             zrow = norm_pool.tile([1, 2, W], F32, tag="zrow", name="zrow")
             zri = norm_pool.tile([1, 2, W], F32, tag="zri", name="zri")
             nc.sync.dma_start(out=zrow[0:1], in_=ytu[64:65])
             # recip BEFORE the deferred-mul flush: it feeds the next
             # window's broadcast->AV chain, so it must not queue behind
             # two slack multiplies in the in-order DVE stream
             nc.vector.reciprocal_approx_fast(
                 zri[0:1].rearrange("p j w -> p (j w)"),
                 zrow[0:1].rearrange("p j w -> p (j w)"))
             zr = norm_pool.tile([64, 2, W], F32, tag="zr", name="zr")
             # flush the two-pairs-ago normalize multiplies: their broadcast
             # finished a full pair ago, so they enter the DVE queue with
             # resolved waits and can never head-of-line-block it
             while len(pending_c) > 1:
                 pending_c.pop(0)()

             def fin_b():
                 nc.gpsimd.partition_broadcast(
                     zr[:].rearrange("p j w -> p (j w)"),
                     zri[0:1].rearrange("p j w -> p (j w)"))

             def fin_c():
                 # head j=0 lives at yt rows 0..64: direct; j=1 needs a
                 # partition shift: stage then DMA.
                 nc.vector.tensor_mul(
                     yt_sb[th][0:64, wbase:wbase + W], ytu[0:64, 0], zr[:, 0])
                 yst = ystage.tile([64, W], md, tag="yst", name="yst")
                 nc.vector.tensor_mul(yst[:], ytu[0:64, 1], zr[:, 1])
                 nc.sync.dma_start(out=yt_sb[th][64:128, wbase:wbase + W],
                                   in_=yst[:])
             pending_b.append(fin_b)
             pending_c.append(fin_c)
             attn_last_norm[0], attn_last_norm[1] = ytu, zr

         wo_psy = [None]

         def emit_wo_dt(sc, dt, ceng=None):
             # one [128 dims, 512 queries] output-projection chunk
             if ceng == "mix" and dt % 4 >= 2:
                 # final chunk: borrow the (now idle) psY banks so four po
                 # tiles rotate instead of two - the PE then streams through
                 # the copies instead of stop-starting at mid pstate
                 if dt % 4 == 2:
                     wo_psy[0] = psY.tile([128, 2, 512], F32, tag="py",
                                          name="powo")
                 po = wo_psy[0][:, dt % 2]
             else:
                 po = pp.tile([128, 512], F32, tag="ppE" if dt % 2 == 0 else "ppO",
                              name="po")
             for t in range(2):
                 nc.tensor.matmul(
                     out=po[:],
                     lhsT=wot[:, t, dt * 128:(dt + 1) * 128],
                     rhs=yt_sb[t][:, sc * 512:(sc + 1) * 512],
                     start=(t == 0), stop=(t == 1),
                 )
             ot = ostage.tile([128, 512], F16, tag="ot", name="ot")
             if ceng == "mix":
                 # pair each copy engine with its own DMA queue (a scalar-
                 # queue trigger waiting on a VECTOR copy would head-of-
                 # line-block the next scalar copy); HWDGE only, so the
                 # end-of-program drain never waits on the slow SWDGE
                 if dt % 2 == 0:
                     nc.scalar.copy(ot[:], po[:])
                     eng = nc.scalar
                 else:
                     nc.vector.tensor_copy(ot[:], po[:])
                     eng = nc.sync
             elif ceng is None:
                 nc.vector.tensor_copy(ot[:], po[:])
                 # keep outputs off the slow SWDGE near the end of the
                 # stream: its backlog otherwise delays the last window's
                 # broadcast and the final drain
                 eng = nc.gpsimd if (dt % 2 == 0 and sc < 1) else nc.sync
             else:
                 ceng.copy(ot[:], po[:])
                 eng = nc.gpsimd if (dt % 2 == 0 and sc < 1) else nc.sync
             eng.dma_start(out=out_d.ap()[dt, :, sc * 512:(sc + 1) * 512],
                           in_=ot[:])

         def emit_wo(sc, ceng=None):
             for dt in range(KT):
                 emit_wo_dt(sc, dt, ceng)

         # ---- emission order --------------------------------------------
         # Window w needs q from chunk sc=w and k/v through chunk w, so
         # q/k pairs and v-tiles interleave one chunk ahead of the window
         # stream; wo for chunk sc streams once both yt halves are final.
         emit_warm(14)
         emit_qk_pair(0, 0)
         emit_warm(10)
         # the k-projection borrows the (idle-until-AV) psY bank pair so it
         # doesn't serialize behind the q-pair's pp rotation at startup
         emit_qk_pair(0, 1, pool=psY, ptag="py")
         nc.sync.dma_start(out=wvt[:], in_=wvt_d.ap()[:])
         if not causal:
             nc.scalar.dma_start(out=xt[:, 2:3], in_=xt_d.ap()[:, 2:3])
             nc.scalar.dma_start(out=xt[:, 3:4], in_=xt_d.ap()[:, 3:4])
             nc.sync.dma_start(out=wot[:], in_=wot_d.ap()[:])
             for st in range(16):
                 emit_v(st)
             for w in range(NW):
                 if w + 1 < NW:
                     emit_qk_pair(w + 1, 0)
                 emit_attn_pair(0, w)
                 if w + 1 < NW:
                     emit_qk_pair(w + 1, 1)
                 emit_attn_pair(1, w)
                 if w >= 1:
                     emit_wo(w - 1)
         else:
             def pre0(w, fill=None):
                 def f(k, vb=4 * w, sc=w - 2, fl=fill):
                     if k < 4:
                         emit_v(vb + k)
                     elif fl:
                         fl.pop(0)()
                     elif sc >= 0 and k < 12:
                         emit_wo_dt(sc, k - 4)
                 return f

             def fill_pre(fill):
                 def f(k, fl=fill):
                     if fl:
                         fl.pop(0)()
                 return f

             # Both sc1 pairs go ahead of the attention stream: their
             # projection matmuls fill the PE during the sc0 rope hops (no
             # >3.4us idle -> HAM stays at full clock through the startup
             # chain) and the sc1 k-shuffles land before the exp stream
             # finishes window 0 (was an 11.9us scalar stall).
             emit_qk_pair(1, 0)
             # the 18-26us window is HBM-bound (wqk-k + xt1 still landing):
             # anchored warm bridges the PE so the clock gate stays hot
             emit_warm(26, anchor=qhat[0][:, 0:128])
             emit_qk_pair(1, 1)
             # late bulk triggers: queued only once the startup-critical
             # loads and the early shuffles are already in their queues
             nc.scalar.dma_start(out=xt[:, 2:3], in_=xt_d.ap()[:, 2:3])
             emit_attn_pair(0, 0, pre_k=pre0(0))
             emit_qk_pair(2, 0)
             nc.scalar.dma_start(out=xt[:, 3:4], in_=xt_d.ap()[:, 3:4])
             emit_attn_pair(1, 0, pre_k=fill_pre(emit_qk_pair_slices(2, 1)))
             nc.sync.dma_start(out=wot[:], in_=wot_d.ap()[:])
             emit_attn_pair(0, 1, pre_k=pre0(1, fill=emit_qk_pair_slices(3, 0)))
             emit_attn_pair(1, 1, pre_k=fill_pre(emit_qk_pair_slices(3, 1)))
             emit_attn_pair(0, 2, pre_k=pre0(2))
             emit_attn_pair(1, 2)
             emit_attn_pair(0, 3, pre_k=pre0(3))
             # the wo(2) interleave below reads yt window 2, so all deferred
             # normalize work must be emitted first (w3 has no projection
             # fills, so an early broadcast costs its gpsimd queue nothing)
             while pending_b:
                 pending_b.pop(0)()
             while pending_c:
                 pending_c.pop(0)()
             emit_attn_pair(1, 3, pre_k=lambda k: emit_wo_dt(2, k - 4)
                            if 4 <= k < 12 else None)
         while pending_b:
             pending_b.pop(0)()
         while pending_c:
             pending_c.pop(0)()
         la = attn_last_at[0]
         if la is not None:
             # staged clock-keeping through the final normalize chain: each
             # stage anchors on successively later data so the PE never
             # idles >1-2us before the last wo runs
             emit_warm(16, anchor=la[:, 0, 0, 0:128], lhsT=warm8[:, 0:128])
             ytu_l, zr_l = attn_last_norm
             if ytu_l is not None:
                 emit_warm(8, anchor=yt_sb[0][0:64, S - W:S - W + 128],
                           lhsT=yt_sb[0][0:64, S - W:S - W + 64])
                 emit_warm(10, anchor=ytu_l[0:64, 0, 0:128],
                           lhsT=ytu_l[0:64, 0, 0:64])
                 emit_warm(10, anchor=zr_l[:, 0, 0:128],
                           lhsT=zr_l[:, 0, 0:64])
         emit_wo(NW - 1, ceng="mix")

    nc.compile()
    return nc


def _get_program(causal: bool, md=MM_DT):
    key = (causal, md)
    if key not in _programs:
        _programs[key] = _build_program(causal, md=md)
    return _programs[key]


def _host_prep(x, freqs_cis, wqkv, wo, md=MM_DT):
    """Build per-core device input arrays."""
    nd = _np_mm_dt(md)
    x = np.ascontiguousarray(np.asarray(x, np.float32))
    freqs_cis = np.asarray(freqs_cis, np.float32)
    wqkv = np.asarray(wqkv, np.float32)
    wo = np.asarray(wo, np.float32)

    # x[b]^T in [128, kt, S] layout
    xts = []
    for b in range(B):
        xt = x[b].T  # [DIM, S]
        # [128, sc, kt, 512]: per-partition contiguous 8KB per s-chunk
        xts.append(np.ascontiguousarray(
            xt.reshape(KT, 128, 4, 512).transpose(1, 2, 0, 3).astype(nd)))

    cosT = np.ascontiguousarray(freqs_cis[:, :, 0].T)  # [32, S]
    sinT = np.ascontiguousarray(freqs_cis[:, :, 1].T)
    cosA = np.ascontiguousarray(np.tile(cosT, (4, 1))).astype(nd)  # [128, S]
    sinA = np.ascontiguousarray(np.tile(sinT, (4, 1))).astype(nd)
    trilm = np.triu(np.ones((128, 128), np.float32)).astype(nd)
    tril2 = np.ascontiguousarray(np.concatenate([trilm, trilm], axis=1))

    Wq, Wk, Wv = wqkv[0:DIM], wqkv[DIM:2 * DIM], wqkv[2 * DIM:3 * DIM]
    wqk_g, wvt_g, wot_g = [], [], []
    for g in range(4):
        heads = range(4 * g, 4 * g + HPC)
        rows_E = [h * HD + 2 * i for h in heads for i in range(32)]
        rows_O = [h * HD + 2 * i + 1 for h in heads for i in range(32)]
        wq = np.concatenate([Wq[rows_E], Wq[rows_O]], axis=0)  # [256, DIM]
        wk = np.concatenate([Wk[rows_E], Wk[rows_O]], axis=0)
        # [128, qk, kt, 256]: per-partition contiguous 4KB per q/k half
        wqkt = np.stack(
            [m.T.reshape(KT, 128, 256).transpose(1, 0, 2) for m in (wq, wk)],
            axis=1)
        wqk_g.append(np.ascontiguousarray(wqkt.astype(nd)))

        rows_v = [h * HD + d for h in heads for d in range(HD)]
        wvt = Wv[rows_v].T.reshape(KT, 128, 256).transpose(1, 0, 2)
        wvt_g.append(np.ascontiguousarray(wvt.astype(nd)))

        wot = wo[:, rows_v].T.reshape(2, 128, 1024).transpose(1, 0, 2)
        wot_g.append(np.ascontiguousarray(wot.astype(nd)))

    in_maps = []
    for c in range(N_CORES):
        b, g = c // 4, c % 4
        in_maps.append({
            "xt": xts[b], "wqkt": wqk_g[g], "wvt": wvt_g[g], "wot": wot_g[g],
            "cosA": cosA, "sinA": sinA, "tril2": tril2,
        })
    return in_maps


def _host_fallback(x, freqs_cis, mask, wqkv, wo):
    """Generic-mask reference path (numpy, chunked over heads)."""
    x = np.asarray(x, np.float64)
    fc = np.asarray(freqs_cis, np.float64)
    m = np.asarray(mask, bool)[0, 0]
    wqkv64 = np.asarray(wqkv, np.float64)
    wo64 = np.asarray(wo, np.float64)
    qkv = x @ wqkv64.T
    q, k, v = np.split(qkv, 3, axis=-1)
    q = q.reshape(B, S, N_HEAD, HD)
    k = k.reshape(B, S, N_HEAD, HD)
    v = v.reshape(B, S, N_HEAD, HD)

    def rope(t):
        ts = t.reshape(*t.shape[:-1], HD // 2, 2)
        cr = fc[None, :, None, :, 0]
        ci = fc[None, :, None, :, 1]
        xr, xi = ts[..., 0], ts[..., 1]
        return np.stack([xr * cr - xi * ci, xi * cr + xr * ci],
                        axis=-1).reshape(t.shape)

    q, k = rope(q), rope(k)
    out = np.zeros((B, S, DIM), np.float64)
    for h in range(N_HEAD):
        sc = np.einsum("bqd,bkd->bqk", q[:, :, h], k[:, :, h]) * (HD ** -0.5)
        sc = np.where(m[None], sc, -np.inf)
        sc -= sc.max(axis=-1, keepdims=True)
        e = np.exp(sc)
        attn = e / e.sum(axis=-1, keepdims=True)
        y = np.einsum("bqk,bkd->bqd", attn, v[:, :, h])
        out += y @ wo64[:, h * HD:(h + 1) * HD].T
    return out.astype(np.float32)


def kernel(x, freqs_cis, mask, wqkv, wo):
    mask_sq = np.asarray(mask, bool)[0, 0]
    if np.array_equal(mask_sq, np.tril(np.ones((S, S), bool))):
        causal = True
    elif mask_sq.all():
        causal = False
    else:
        return _host_fallback(x, freqs_cis, mask, wqkv, wo)

    # bf16 operands are plenty for genuine rotary tables (cos^2+sin^2=1);
    # free-form freqs widen the logit range beyond bf16 comfort, so take the
    # exact host path for that (not expected in practice).
    fc = np.asarray(freqs_cis, np.float32)
    if not np.allclose(fc[..., 0] ** 2 + fc[..., 1] ** 2, 1.0, atol=0.2):
        return _host_fallback(x, freqs_cis, mask, wqkv, wo)
    md = BF16
    nc = _get_program(causal, md)
    in_maps = _host_prep(x, freqs_cis, wqkv, wo, md)
    res = run_bass_kernel_spmd(nc, in_maps, core_ids=list(range(N_CORES)))

    out = np.zeros((B, S, DIM), np.float32)
    for c in range(N_CORES):
        b = c // 4
        out[b] += res.results[c]["outp"].reshape(DIM, S).T.astype(np.float32)
    return out



# revision 69
# speedup vs baseline: 1.0218x; 1.0038x over previous
"""Multi-head causal attention (B=2, S=2048, D=1024, 16 heads x 64) on 8 trn2
NeuronCores.

Sharding: core c = 4*b + g handles batch b and heads [4g, 4g+4) (tensor
parallel over heads, data parallel over batch). Each core:
  - projects q/k/v for its heads from x[b] (wqkv column-sharded by head),
  - applies rotary embeddings,
  - computes causal softmax(q k^T / sqrt(d)) v in a transposed-score layout,
  - multiplies by its shard of wo^T to produce a partial [D, S] output (fp16).
The host sums the 4 head-group partials per batch and transposes.

Device-side layouts (per core):
  xt      [128, 4, 8, 512]  x[b]^T, s-chunk-major: [partition, s-chunk,
                          k-tile, 512 queries] so each 1MB s-chunk is one
                          contiguous DMA and window 0's projection does not
                          wait for the full 4MB load (~100-110GB/s per queue)
  wqkt    [128, 2, 8, 256]  W_{q,k}^T as [partition, q/k, k-tile, 4 heads x
                          (32 evens | 32 odds)] so RoPE runs as full-width
                          vector ops and q/k halves are contiguous DMAs
  wvt     [128, 8, 256]   W_v^T, natural head-dim order
  wot     [128, 2, 1024]  wo[:, head cols]^T (matmul stationary)
  cosA/sinA [128, 2048]   rotary tables tiled 4x over the 32 pair dims
  tril2   [128, 256]      upper-triangular 0/1 x2 (valid = key <= query);
                          cast once on device to tril8 (fp8) for the fp8 at
  qhat/khat [th][128, S]  packed head pairs: rows 64j..64j+64 = head 2th+j
                          as [evens(32); odds(32)], bf16
  v8_sb   [128,16,4,80]   fp8 v, per s-tile per head slot [v(64)|ones|pad];
                          the 80-slot keeps the DoubleRow weights AP slab
                          stride (4*80) a multiple of 16 bytes
  vb_sb   [128,2,4,65]    exact bf16 v for s-tiles 0,1 only
  at      [128,2,2,512]   exp output per (head pair, K-TILE PAIR): fp8, dims
                          [key, k-tile slab, head, query]
  outp    [8, 128, 2048]  partial output, d-major, fp16

Precision strategy (metric = max|err|/absmax ~ 0.75x rms_rel; budget 2e-2):
q/k/v projections, rope, scores and wo all stay bf16 - fp8 anywhere on
those paths puts ~5-10% relative noise on y (softmax does NOT attenuate
relative error: y shrinks with attention entropy exactly as fast as the
noise). The ONE fp8 win that survives: the AV contraction, as
perf_mode=DoubleRow over K=256 (two 128-key tiles packed in the at tile's
slab dim, v8 slab stride 320B), HW-measured at 2x bf16 throughput. Early
queries (low entropy, y near full variance, dominate the max-err metric)
are protected by running window 0's first k-tile pair through an exact
bf16 two-matmul AV (queries 0..255 then see zero fp8 noise). exp runs
with bias=-2 so e^x stays inside fp8e4's 240 max; the bias cancels in the
softmax ratio since the ones-row denominator sees the same factor.
Measured rel_err 4.1e-3 (bf16 baseline was 3.8e-3).

Scores use K=64 matmuls (tile_position row groups 0/64) so the two heads
of a pair run concurrently on the PE array halves; each (pair, k-tile)
produces a two-head-wide [128, 2x512] PSUM tile consumed by ONE wide exp
ACTIVATE writing the fp8 at slab. ACTIVATE cost is free-size x 0.83ns
regardless of dtype, so keys stay on all 128 partitions and the exp
stream (~65-80us total) is the hard scalar-engine floor. For a diagonal
k-tile pair the odd slab's pre-qs query range is never written by its
exp but is summed by the DR matmul: a gpsimd memset zeroes the gap
(pool rotation leaves stale data there, not zeros).

Schedule: both sc1 projection pairs are front-loaded; anchored warm
matmuls (rhs pinned on already-loaded data so the Tile scheduler cannot
float them ahead) bridge the HBM-bound 14-26us startup window - the first
20us of input DMA is at the 2x ~105GB/s HWDGE roofline, so xt0/wqk/cos/
sin/xt1 split need-ordered across the sync+scalar queues (xt1 half on
each), and xt2/xt3/wot triggers are emitted LATE (mid-pipeline) so their
bulk never sits ahead of the latency-critical rope-shuffle DMAs. gpsimd
(SWDGE, ~3x slower, software FIFO) carries no bulk input and loses the
output stores from chunk >= 1 (its backlog otherwise delays the final
drain by ~5us). The scalar queue carries ONLY the exp stream plus
startup/late input triggers.

The per-pair softmax normalize is SPLIT AND DEFERRED to keep the in-order
DVE/gpsimd queues from head-of-line-blocking the AV critical path
(semaphore waits transitively stall everything behind them in a queue):
psum evict + fp32 reciprocal emit at pair end; the gpsimd partition-
broadcast of 1/z emits at the NEXT pair's m==nm-2 (after its fills'
shuffles are queued); the yt multiplies + yst partition-shift DMA emit
TWO pairs later (their broadcast is then guaranteed complete, so they
enter the DVE queue wait-free). Everything pending flushes before the
wo(2) interleave in the last pair (which reads yt window 2) and again at
the tail. Rope ops stage the projection psum to bf16 first: the six
rotate ops then run in DVE 2x mode and the pp psum banks free after two
copies instead of four reads. wo output chunks stream as per-dt slivers
through the pre_k hooks of later attention passes; the final chunk
borrows the idle psY banks so four po tiles rotate (PE streams through
the copies at full pstate) and its stores ride sync+scalar only. Tail
warm matmuls are staged on successively later anchors (last at tile ->
yt -> ytu -> zr) so the PE never idles >2us through the final normalize
chain and the last wo runs at speed.

Beware the power-state lottery: the chip drops ALL engine clocks ~17%
(PE 2.4->2.0GHz) under sustained load, stickily across runs (PE busy in
the trace inflates 128->152us with unchanged instruction count). Verify
the clock via back-to-back N=512 matmul deltas (216ns warm vs 259ns)
before comparing timings. Measured fast-state: 163.4us (bf16 baseline
191.0us); do not trust single-run deltas under ~5us.

Things tried that did NOT work (traced, reverted):
  - fp8 DoubleRow for projections/scores: numerically dead (see above).
  - tril mask multiplies on gpsimd: its FIFO serialized the AV path
    behind broadcasts/shuffles (12-15us window gaps).
  - on-device cos/sin 4x partition dup via SBUF-SBUF DMA: correct in
    isolation, corrupts under load in-kernel (suspected partition-region
    dependency tracking); host-tiled [128,S] kept instead.
  - scores as 4 concurrent 32-row DoubleRow matmuls: AP base_partition
    is ISA-limited to {0,32,64}; row 96 is unreachable.
"""

import numpy as np
import ml_dtypes

import concourse.bass as bass
import concourse.mybir as mybir
import concourse.tile as tile
from concourse import bacc
from concourse.bass_utils import run_bass_kernel_spmd

N_CORES = 8
B, S, DIM = 2, 2048, 1024
N_HEAD, HD = 16, 64
HPC = N_HEAD // 4  # heads per core = 4
KT = DIM // 128  # 8 contraction tiles over model dim
F32 = mybir.dt.float32
F16 = mybir.dt.float16
BF16 = mybir.dt.bfloat16
FP8 = mybir.dt.float8e4
DR = mybir.MatmulPerfMode.DoubleRow
MM_DT = BF16
W = 512  # query window width
NW = S // W  # 4 windows
VSLOT = 80   # fp8 v columns per head slot ([v(64) | ones | pad]; 16B-aligned
             # so the DoubleRow weights AP slab stride (4*VSLOT) is %16)
VTILE = HPC * VSLOT  # 320 fp8 v columns per s-tile
EXP_BIAS = -2.0  # logits bias before exp: keeps e^x inside fp8e4 range
                 # (max finite 240); cancels exactly in the softmax ratio

_programs = {}


def _np_mm_dt(md):
    return ml_dtypes.bfloat16 if md == BF16 else np.float32


def _build_program(causal: bool, md=MM_DT):
    nc = bacc.Bacc("TRN2", target_bir_lowering=False, debug=False,
                   num_devices=N_CORES)

    xt_d = nc.dram_tensor("xt", [128, 4, KT, 512], md, kind="ExternalInput")
    wqkt_d = nc.dram_tensor("wqkt", [128, 2, KT, 256], md, kind="ExternalInput")
    wvt_d = nc.dram_tensor("wvt", [128, KT, 256], md, kind="ExternalInput")
    wot_d = nc.dram_tensor("wot", [128, 2, 1024], md, kind="ExternalInput")
    cos_d = nc.dram_tensor("cosA", [128, S], md, kind="ExternalInput")
    sin_d = nc.dram_tensor("sinA", [128, S], md, kind="ExternalInput")
    tril_d = nc.dram_tensor("tril2", [128, 256], md, kind="ExternalInput")
    out_d = nc.dram_tensor("outp", [KT, 128, S], F16, kind="ExternalOutput")

    with tile.TileContext(nc) as tc:
      with (
        tc.tile_pool(name="persist", bufs=1) as persist,
        tc.tile_pool(name="pha", bufs=1) as pha,
        tc.tile_pool(name="rope_out", bufs=4) as rope_out,
        tc.tile_pool(name="rope_tmp", bufs=3) as rope_tmp,
        tc.tile_pool(name="attn", bufs=4) as attn_pool,
        tc.tile_pool(name="attnb", bufs=2) as attnb_pool,
        tc.tile_pool(name="norm", bufs=3) as norm_pool,
        tc.tile_pool(name="ystage", bufs=2) as ystage,
        tc.tile_pool(name="ostage", bufs=3) as ostage,
        tc.tile_pool(name="psS", bufs=2, space="PSUM") as psS,
        tc.tile_pool(name="psY", bufs=1, space="PSUM") as psY,
        tc.tile_pool(name="pp", bufs=1, space="PSUM") as pp,
      ):
         # packed head-pair tiles: rows 64j.. = head 2th+j as [E32; O32]
         qhat = [persist.tile([128, S], md, tag=f"qhat{t}", name=f"qhat{t}") for t in range(2)]
         khat = [persist.tile([128, S], md, tag=f"khat{t}", name=f"khat{t}") for t in range(2)]
         # fp8 v for the DoubleRow AV path: 16 s-tiles x 4 slots of
         # [v(64) | ones | pad(15)]
         v8_sb = persist.tile([128, 16, HPC, VSLOT], FP8, tag="v8_sb")
         # exact bf16 v for s-tiles 0,1 (window-0 first k-pair runs bf16 so
         # queries 0..255 see no fp8 noise; their y is near full variance and
         # dominates the max-err metric)
         vb_sb = persist.tile([128, 2, HPC, HD + 1], md, tag="vb_sb")
         yt_sb = [persist.tile([128, S], md, tag=f"yt{t}", name=f"yt{t}") for t in range(2)]
         tril_sb = persist.tile([128, 256], md, tag="tril")
         tril8 = persist.tile([128, 256], FP8, tag="tril8")
         nbias = persist.tile([128, 1], F32, tag="nbias")
         wot = persist.tile([128, 2, 1024], md, tag="wot")
         warm_sb = persist.tile([128, 512], md, tag="warm")
         warm8 = persist.tile([128, 128], FP8, tag="warm8")
         xt = pha.tile([128, 4, KT, 512], md, tag="xt")
         wqk = pha.tile([128, 2, KT, 256], md, tag="wqk")
         wvt = pha.tile([128, KT, 256], md, tag="wvt")
         cosA = pha.tile([128, S], md, tag="cos")
         sinA = pha.tile([128, S], md, tag="sin")

         nc.vector.memset(warm_sb[:], 0.0)
         nc.vector.memset(warm8[:], 0.0)
         nc.vector.memset(nbias[:], EXP_BIAS)
         # ---- input DMAs. Aggregate HBM read is ~200GB/s with all 8 cores
         # pulling at once, so the load order IS the startup critical path.
         # xt goes s-chunk-major (window 0's projection needs only s-chunk 0,
         # all k-tiles); the first projection transitively needs just the
         # wqk-q half + xt-s0 (1.5MB), so those split across all four HWDGE
         # queues to land in parallel before anything else.
         # sync + scalar are HWDGE (~100GB/s each); gpsimd is SWDGE and its
         # queue must stay clear for the latency-critical rope shuffles, so
         # it carries no bulk. The first ~20us of HBM reads are roofline-
         # critical: only what the front of the pipeline needs goes first
         # (xt0+wqk+cos/sin+xt1+wvt ~= 3.75MB ~= both queues' 20us budget).
         # cos/sin come in untiled [32,S] and are 4x-duplicated on device.
         # xt2/xt3/wot triggers are emitted LATER, mid-pipeline, so their
         # bulk never sits ahead of shuffles in any queue.
         nc.sync.dma_start(out=xt[:, 0, 0:4], in_=xt_d.ap()[:, 0, 0:4])
         nc.scalar.dma_start(out=wqk[:, 0:1], in_=wqkt_d.ap()[:, 0:1])
         nc.sync.dma_start(out=cosA[:], in_=cos_d.ap()[:])
         nc.scalar.dma_start(out=xt[:, 0, 4:8], in_=xt_d.ap()[:, 0, 4:8])
         nc.gpsimd.dma_start(out=tril_sb[:], in_=tril_d.ap()[:])
         nc.sync.dma_start(out=wqk[:, 1:2], in_=wqkt_d.ap()[:, 1:2])
         nc.sync.dma_start(out=sinA[:], in_=sin_d.ap()[:])
         nc.scalar.dma_start(out=xt[:, 1:2], in_=xt_d.ap()[:, 1:2])
         # wvt's trigger is emitted AFTER the sc0 q/k ropes (below) so its
         # bulk sits behind their shuffle DMAs in the sync queue, not ahead
         nc.vector.tensor_copy(tril8[:], tril_sb[:])
         # ones rows (slot column 64) for the AV denominator, one strided
         # memset over all tiles/slots each
         nc.vector.memset(v8_sb[:, :, :, HD:HD + 1], 1.0)
         nc.vector.memset(vb_sb[:, :, :, HD:HD + 1], 1.0)

         attn_last_at = [None]
         attn_last_norm = [None, None]
         pending_av = []  # deferred last AV + psum evict of each pair
         pending_b = []   # deferred normalize: gpsimd broadcast
         pending_c = []   # deferred normalize: yt multiplies + yst DMA

         # ---- emission helpers ------------------------------------------
         def emit_warm(n, anchor=None, lhsT=None, width=512):
             # dummy matmuls with no DMA dependencies: keep the PE busy
             # through input-DMA pacing gaps so the HAM clock gate stays
             # at full speed (idle windows drop the PE to half clock).
             # An anchor rhs pins them against the Tile scheduler floating
             # them ahead of the gap they are meant to bridge; short widths
             # give fine-grained bridges that overshoot less into real work.
             wu = psS.tile([128, 1024], F32, tag="psS", name="wu")
             rhs = warm_sb[:, 0:width] if anchor is None else anchor
             lt = warm_sb[:, 0:128] if lhsT is None else lhsT
             for i in range(n):
                 nc.tensor.matmul(out=wu[0:lt.shape[-1], 0:rhs.shape[-1]],
                                  lhsT=lt, rhs=rhs,
                                  start=(i == 0), stop=(i == n - 1))

         def emit_qk_proj(sc, qk, eo, pt):
             for kt in range(KT):
                 nc.tensor.matmul(
                     out=pt[:],
                     lhsT=wqk[:, qk, kt, eo * 128:(eo + 1) * 128],
                     rhs=xt[:, sc, kt, :],
                     start=(kt == 0), stop=(kt == KT - 1),
                 )
                 if sc == 0 and qk == 0 and eo == 0 and kt == 3:
                     # xt kt4-7 and the k/v weights are still in flight on
                     # the HWDGE queues here; bridge the PE on loaded data
                     emit_warm(12, anchor=xt[:, 0, 0, 0:128])

         def emit_qk_pair_slices(sc, qk):
             """The projection pair as 3 thunks (proj-E, proj-O, rope+shuffle)
             drained one-per-k-iter through an attention pass's pre_k hook, so
             the 16-MM block never dams the in-order PE queue ahead of the
             next window's score matmuls. The ppE/ppO tile allocation happens
             in the first thunk: no other ppE-tag user may be emitted between
             the thunks (pool rotation + PE FIFO would deadlock).
             """
             st = {}

             def ensure():
                 if "pE" not in st:
                     st["pE"] = pp.tile([128, 512], F32, tag="ppE", name="ppE")
                     st["pO"] = pp.tile([128, 512], F32, tag="ppO", name="ppO")

             def s_eo(eo):
                 def f():
                     ensure()
                     emit_qk_proj(sc, qk, eo, st["pE"] if eo == 0 else st["pO"])
                 return f

             def fin():
                 emit_rope(sc, qk, st["pE"], st["pO"])
             return [s_eo(0), s_eo(1), fin]

         def emit_qk_pair(sc, qk, pool=None, ptag=None):
             """Project + rope one (s-chunk, q-or-k) pair of e-tiles."""
             if pool is None:
                 pE = pp.tile([128, 512], F32, tag="ppE", name="ppE")
                 pO = pp.tile([128, 512], F32, tag="ppO", name="ppO")
             else:
                 pEO = pool.tile([128, 2, 512], F32, tag=ptag, name="ppEO")
                 pE, pO = pEO[:, 0], pEO[:, 1]
             for eo, pt in ((0, pE), (1, pO)):
                 emit_qk_proj(sc, qk, eo, pt)
             emit_rope(sc, qk, pE, pO)

         def emit_rope(sc, qk, pE, pO):
             cs = cosA[:, sc * 512:(sc + 1) * 512]
             sn = sinA[:, sc * 512:(sc + 1) * 512]
             oE = rope_out.tile([128, 512], md, tag="ropeE", name="ropeE")
             oO = rope_out.tile([128, 512], md, tag="ropeO", name="ropeO")
             pb = rope_tmp.tile([128, 2, 512], md, tag="ropepb", name="ropepb")
             tmp = rope_tmp.tile([128, 512], md, tag="ropetmp", name="ropetmp")
             # stage the psum fp32 down to bf16 first: the six rotate ops
             # then run all-SBUF/2-byte (DVE 2x mode) and the projection
             # psum banks free after two copies instead of four reads
             nc.vector.tensor_copy(pb[:, 0], pE[:])
             nc.vector.tensor_copy(pb[:, 1], pO[:])
             pEb, pOb = pb[:, 0], pb[:, 1]
             # oE = pE*cos - pO*sin ; oO = pO*cos + pE*sin
             nc.vector.tensor_mul(tmp[:], pOb, sn)
             nc.vector.tensor_mul(oE[:], pEb, cs)
             nc.vector.tensor_sub(oE[:], oE[:], tmp[:])
             nc.vector.tensor_mul(tmp[:], pEb, sn)
             nc.vector.tensor_mul(oO[:], pOb, cs)
             nc.vector.tensor_add(oO[:], oO[:], tmp[:])
             for h in range(HPC):
                 r0 = (h % 2) * 64
                 dst = qhat[h // 2] if qk == 0 else khat[h // 2]
                 eng = nc.sync if h % 2 == 0 else nc.gpsimd
                 eng.dma_start(out=dst[r0:r0 + 32, sc * 512:(sc + 1) * 512],
                               in_=oE[32 * h:32 * h + 32, :])
                 eng.dma_start(out=dst[r0 + 32:r0 + 64, sc * 512:(sc + 1) * 512],
                               in_=oO[32 * h:32 * h + 32, :])

         def emit_v(st):
             pv = pp.tile([128, 256], F32, tag="ppE", name="pv")
             for kt in range(KT):
                 nc.tensor.matmul(
                     out=pv[:],
                     lhsT=xt[:, st // 4, kt, (st % 4) * 128:(st % 4 + 1) * 128],
                     rhs=wvt[:, kt, :],
                     start=(kt == 0), stop=(kt == KT - 1),
                 )
             pvh = pv[:].rearrange("p (h d) -> p h d", h=HPC)
             nc.vector.tensor_copy(v8_sb[:, st, :, 0:HD], pvh)
             if st < 2:
                 nc.vector.tensor_copy(vb_sb[:, st, :, 0:HD], pvh)

         def emit_attn_pair(th, w, pre_k=None):
             """Attention for head pair th on query window [wbase, wbase+W).

            Per k-tile: two K=64 score matmuls (one per head, PE row groups
            0/64, concurrent), ONE wide exp over both heads' scores (out in
            fp8, bias EXP_BIAS), tril mask, then per k-tile PAIR one fp8
            DoubleRow AV matmul per head (K=256 over two key tiles packed as
            the at tile's slab dim). Window 0's first pair runs the exact
            bf16 two-matmul AV instead (early queries' y is near full
            variance; fp8's ~4% relative noise there would break the max-err
            budget). Software-pipelined: scores(pair m+1) is emitted before
            AV(m) so the PE streams during the exp.
             """
             wbase = w * W
             kmax = (wbase + W) // 128 if causal else 16
             nm = kmax // 2  # k-tile pairs
             pyh = {}  # py psum allocated lazily at the first AV: the
             # previous pair's deferred last-AV/evict must be emitted first
             # so the psY rotation sees its readers
             ats = {}

             def emit_scores(k):
                 if pre_k is not None:
                     pre_k(k)
                 bfp = causal and w == 0 and k < 2  # exact-path pair
                 qs = max(wbase, 128 * k) - wbase if causal else 0
                 m, kk = k // 2, k % 2
                 pscore = psS.tile([128, 2, W], F32, tag="psS", name="psS")
                 for j in range(2):
                     nc.tensor.matmul(
                         out=pscore[:, j, qs:W],
                         lhsT=khat[th][64 * j:64 * j + 64, k * 128:(k + 1) * 128],
                         rhs=qhat[th][64 * j:64 * j + 64, wbase + qs:wbase + W],
                         start=True, stop=True,
                     )
                 if kk == 0:
                     if bfp:
                         at = attnb_pool.tile([128, 2, 2, W], md, tag="atb",
                                              name="atb")
                     else:
                         at = attn_pool.tile([128, 2, 2, W], FP8, tag="at",
                                             name="at")
                     ats[m] = [at, qs]
                     gs = max(wbase, 128 * (k + 1)) - wbase if causal else 0
                     if not bfp and gs > qs:
                         # slab 1's pre-qs queries are never written by its
                         # exp but are summed by the DR matmul: zero the gap
                         # (stale data from the pool's previous rotation)
                         nc.gpsimd.memset(at[:, 1, :, qs:gs], 0.0)
                 at = ats[m][0]
                 attn_last_at[0] = at
                 nc.scalar.activation(
                     at[:, kk, :, qs:W], pscore[:, :, qs:W],
                     mybir.ActivationFunctionType.Exp,
                     scale=float(HD) ** -0.5, bias=nbias[:])
                 if causal and 128 * k >= wbase:
                     trl = tril_sb if bfp else tril8
                     nc.vector.tensor_mul(
                         at[:, kk, :, qs:qs + 128], at[:, kk, :, qs:qs + 128],
                         trl[:].rearrange("p (j w) -> p j w", j=2))

             def emit_av(m):
                 if m == 0:
                     pyh['py'] = psY.tile([128, 2, W], F32, tag="py",
                                          name="py")
                 py = pyh['py']
                 at, qs = ats.pop(m)
                 bfp = causal and w == 0 and m == 0
                 for j in range(2):
                     if bfp:
                         for kk in range(2):
                             qk = max(wbase, 128 * (2 * m + kk)) - wbase
                             nc.tensor.matmul(
                                 out=py[0:HD + 1, j, qk:W],
                                 lhsT=vb_sb[:, 2 * m + kk, 2 * th + j, :],
                                 rhs=at[:, kk, j, qk:W],
                                 start=(m == 0 and kk == 0), stop=False,
                             )
                     else:
                         nc.tensor.matmul(
                             out=py[0:HD + 1, j, qs:W],
                             lhsT=v8_sb[:, 2 * m:2 * m + 2, 2 * th + j, 0:HD + 1],
                             rhs=at[:, :, j, qs:W],
                             start=(m == 0), stop=(m == nm - 1),
                             perf_mode=DR,
                         )

             for m in range(nm):
                 emit_scores(2 * m)
                 emit_scores(2 * m + 1)
                 if m == 0:
                     # the PREVIOUS pair's last AV + evict flush here, AFTER
                     # this pair's first two score tiles: those scores (and
                     # their exps) then overlap the old AV's streaming in
                     # the in-order PE queue instead of serializing with it
                     while pending_av:
                         pending_av.pop(0)()
                 if m == max(0, nm - 2):
                     # previous pair's broadcast: late enough that this
                     # pair's pre_k fills already queued their rope shuffles
                     # ahead of it on gpsimd
                     while pending_b:
                         pending_b.pop(0)()
                 if m > 0:
                     emit_av(m - 1)

             def last_block():
                 emit_av(nm - 1)
                 py = pyh['py']
                 # Evict psum promptly (gates the psY rotation for the next
                 # pair's AVs). The normalize chain splits onward: recip now
                 # (input ready, short wait), broadcast at the next pair's
                 # m==nm-2 (after its fills' shuffles queue on gpsimd),
                 # multiplies two pairs later (their broadcast then resolved
                 # so they can never head-of-line-block the DVE queue).
                 ytu = norm_pool.tile([65, 2, W], F32, tag="ytu", name="ytu")
                 nc.vector.tensor_copy(ytu[:], py[0:65])
                 zrow = norm_pool.tile([1, 2, W], F32, tag="zrow", name="zrow")
                 zri = norm_pool.tile([1, 2, W], F32, tag="zri", name="zri")
                 nc.sync.dma_start(out=zrow[0:1], in_=ytu[64:65])
                 nc.vector.reciprocal_approx_fast(
                     zri[0:1].rearrange("p j w -> p (j w)"),
                     zrow[0:1].rearrange("p j w -> p (j w)"))
                 zr = norm_pool.tile([64, 2, W], F32, tag="zr", name="zr")
                 while len(pending_c) > 1:
                     pending_c.pop(0)()

                 def fin_b():
                     nc.gpsimd.partition_broadcast(
                         zr[:].rearrange("p j w -> p (j w)"),
                         zri[0:1].rearrange("p j w -> p (j w)"))

                 def fin_c():
                     # head j=0 lives at yt rows 0..64: direct; j=1 needs a
                     # partition shift: stage then DMA.
                     nc.vector.tensor_mul(
                         yt_sb[th][0:64, wbase:wbase + W], ytu[0:64, 0],
                         zr[:, 0])
                     yst = ystage.tile([64, W], md, tag="yst", name="yst")
                     nc.vector.tensor_mul(yst[:], ytu[0:64, 1], zr[:, 1])
                     nc.sync.dma_start(out=yt_sb[th][64:128, wbase:wbase + W],
                                       in_=yst[:])
                 pending_b.append(fin_b)
                 pending_c.append(fin_c)
                 attn_last_norm[0], attn_last_norm[1] = ytu, zr
             pending_av.append(last_block)

         wo_psy = [None]

         def emit_wo_dt(sc, dt, ceng=None):
             # one [128 dims, 512 queries] output-projection chunk
             if ceng == "mix" and dt % 4 >= 2:
                 # final chunk: borrow the (now idle) psY banks so four po
                 # tiles rotate instead of two - the PE then streams through
                 # the copies instead of stop-starting at mid pstate
                 if dt % 4 == 2:
                     wo_psy[0] = psY.tile([128, 2, 512], F32, tag="py",
                                          name="powo")
                 po = wo_psy[0][:, dt % 2]
             else:
                 po = pp.tile([128, 512], F32, tag="ppE" if dt % 2 == 0 else "ppO",
                              name="po")
             for t in range(2):
                 nc.tensor.matmul(
                     out=po[:],
                     lhsT=wot[:, t, dt * 128:(dt + 1) * 128],
                     rhs=yt_sb[t][:, sc * 512:(sc + 1) * 512],
                     start=(t == 0), stop=(t == 1),
                 )
             ot = ostage.tile([128, 512], F16, tag="ot", name="ot")
             if ceng == "mix":
                 # pair each copy engine with its own DMA queue (a scalar-
                 # queue trigger waiting on a VECTOR copy would head-of-
                 # line-block the next scalar copy); HWDGE only, so the
                 # end-of-program drain never waits on the slow SWDGE
                 if dt % 2 == 0:
                     nc.scalar.copy(ot[:], po[:])
                     eng = nc.scalar
                 else:
                     nc.vector.tensor_copy(ot[:], po[:])
                     eng = nc.sync
             elif ceng is None:
                 nc.vector.tensor_copy(ot[:], po[:])
                 # keep outputs off the slow SWDGE near the end of the
                 # stream: its backlog otherwise delays the last window's
                 # broadcast and the final drain
                 eng = nc.gpsimd if (dt % 2 == 0 and sc < 1) else nc.sync
             else:
                 ceng.copy(ot[:], po[:])
                 eng = nc.gpsimd if (dt % 2 == 0 and sc < 1) else nc.sync
             eng.dma_start(out=out_d.ap()[dt, :, sc * 512:(sc + 1) * 512],
                           in_=ot[:])

         def emit_wo(sc, ceng=None):
             for dt in range(KT):
                 emit_wo_dt(sc, dt, ceng)

         # ---- emission order --------------------------------------------
         # Window w needs q from chunk sc=w and k/v through chunk w, so
         # q/k pairs and v-tiles interleave one chunk ahead of the window
         # stream; wo for chunk sc streams once both yt halves are final.
         emit_warm(14)
         emit_qk_pair(0, 0)
         emit_warm(10)
         # the k-projection borrows the (idle-until-AV) psY bank pair so it
         # doesn't serialize behind the q-pair's pp rotation at startup
         emit_qk_pair(0, 1, pool=psY, ptag="py")
         nc.sync.dma_start(out=wvt[:], in_=wvt_d.ap()[:])
         if not causal:
             nc.scalar.dma_start(out=xt[:, 2:3], in_=xt_d.ap()[:, 2:3])
             nc.scalar.dma_start(out=xt[:, 3:4], in_=xt_d.ap()[:, 3:4])
             nc.sync.dma_start(out=wot[:], in_=wot_d.ap()[:])
             for st in range(16):
                 emit_v(st)
             for w in range(NW):
                 if w + 1 < NW:
                     emit_qk_pair(w + 1, 0)
                 emit_attn_pair(0, w)
                 if w + 1 < NW:
                     emit_qk_pair(w + 1, 1)
                 emit_attn_pair(1, w)
                 if w >= 1:
                     while pending_av:
                         pending_av.pop(0)()
                     while pending_b:
                         pending_b.pop(0)()
                     while pending_c:
                         pending_c.pop(0)()
                     emit_wo(w - 1)
         else:
             def pre0(w, fill=None):
                 def f(k, vb=4 * w, sc=w - 2, fl=fill):
                     if k < 4:
                         emit_v(vb + k)
                     elif fl:
                         fl.pop(0)()
                     elif sc >= 0 and k < 12:
                         emit_wo_dt(sc, k - 4)
                 return f

             def fill_pre(fill):
                 def f(k, fl=fill):
                     if fl:
                         fl.pop(0)()
                 return f

             # Both sc1 pairs go ahead of the attention stream: their
             # projection matmuls fill the PE during the sc0 rope hops (no
             # >3.4us idle -> HAM stays at full clock through the startup
             # chain) and the sc1 k-shuffles land before the exp stream
             # finishes window 0 (was an 11.9us scalar stall).
             emit_qk_pair(1, 0)
             # the 18-26us window is HBM-bound (wqk-k + xt1 still landing):
             # anchored warm bridges the PE so the clock gate stays hot
             emit_warm(26, anchor=qhat[0][:, 0:128])
             emit_qk_pair(1, 1)
             # late bulk triggers: queued only once the startup-critical
             # loads and the early shuffles are already in their queues
             nc.scalar.dma_start(out=xt[:, 2:3], in_=xt_d.ap()[:, 2:3])
             emit_attn_pair(0, 0, pre_k=pre0(0))
             emit_qk_pair(2, 0)
             nc.scalar.dma_start(out=xt[:, 3:4], in_=xt_d.ap()[:, 3:4])
             emit_attn_pair(1, 0, pre_k=fill_pre(emit_qk_pair_slices(2, 1)))
             nc.sync.dma_start(out=wot[:], in_=wot_d.ap()[:])
             emit_attn_pair(0, 1, pre_k=pre0(1, fill=emit_qk_pair_slices(3, 0)))
             emit_attn_pair(1, 1, pre_k=fill_pre(emit_qk_pair_slices(3, 1)))
             emit_attn_pair(0, 2, pre_k=pre0(2))
             emit_attn_pair(1, 2)
             emit_attn_pair(0, 3, pre_k=pre0(3))
             # the wo(2) interleave below reads yt window 2, so all deferred
             # work must be emitted first (w3 has no projection fills, so an
             # early broadcast costs its gpsimd queue nothing)
             while pending_av:
                 pending_av.pop(0)()
             while pending_b:
                 pending_b.pop(0)()
             while pending_c:
                 pending_c.pop(0)()
             emit_attn_pair(1, 3, pre_k=lambda k: emit_wo_dt(2, k - 4)
                            if 4 <= k < 12 else None)
         while pending_av:
             pending_av.pop(0)()
         while pending_b:
             pending_b.pop(0)()
         while pending_c:
             pending_c.pop(0)()
         la = attn_last_at[0]
         if la is not None:
             # staged clock-keeping through the final normalize chain: each
             # stage anchors on successively later data so the PE never
             # idles >1-2us before the last wo runs
             emit_warm(16, anchor=la[:, 0, 0, 0:128], lhsT=warm8[:, 0:128])
             ytu_l, zr_l = attn_last_norm
             if ytu_l is not None:
                 emit_warm(8, anchor=yt_sb[0][0:64, S - W:S - W + 128],
                           lhsT=yt_sb[0][0:64, S - W:S - W + 64])
                 emit_warm(10, anchor=ytu_l[0:64, 0, 0:128],
                           lhsT=ytu_l[0:64, 0, 0:64])
                 emit_warm(10, anchor=zr_l[:, 0, 0:128],
                           lhsT=zr_l[:, 0, 0:64])
         emit_wo(NW - 1, ceng="mix")

    nc.compile()
    return nc


def _get_program(causal: bool, md=MM_DT):
    key = (causal, md)
    if key not in _programs:
        _programs[key] = _build_program(causal, md=md)
    return _programs[key]


def _host_prep(x, freqs_cis, wqkv, wo, md=MM_DT):
    """Build per-core device input arrays."""
    nd = _np_mm_dt(md)
    x = np.ascontiguousarray(np.asarray(x, np.float32))
    freqs_cis = np.asarray(freqs_cis, np.float32)
    wqkv = np.asarray(wqkv, np.float32)
    wo = np.asarray(wo, np.float32)

    # x[b]^T in [128, kt, S] layout
    xts = []
    for b in range(B):
        xt = x[b].T  # [DIM, S]
        # [128, sc, kt, 512]: per-partition contiguous 8KB per s-chunk
        xts.append(np.ascontiguousarray(
            xt.reshape(KT, 128, 4, 512).transpose(1, 2, 0, 3).astype(nd)))

    cosT = np.ascontiguousarray(freqs_cis[:, :, 0].T)  # [32, S]
    sinT = np.ascontiguousarray(freqs_cis[:, :, 1].T)
    cosA = np.ascontiguousarray(np.tile(cosT, (4, 1))).astype(nd)  # [128, S]
    sinA = np.ascontiguousarray(np.tile(sinT, (4, 1))).astype(nd)
    trilm = np.triu(np.ones((128, 128), np.float32)).astype(nd)
    tril2 = np.ascontiguousarray(np.concatenate([trilm, trilm], axis=1))

    Wq, Wk, Wv = wqkv[0:DIM], wqkv[DIM:2 * DIM], wqkv[2 * DIM:3 * DIM]
    wqk_g, wvt_g, wot_g = [], [], []
    for g in range(4):
        heads = range(4 * g, 4 * g + HPC)
        rows_E = [h * HD + 2 * i for h in heads for i in range(32)]
        rows_O = [h * HD + 2 * i + 1 for h in heads for i in range(32)]
        wq = np.concatenate([Wq[rows_E], Wq[rows_O]], axis=0)  # [256, DIM]
        wk = np.concatenate([Wk[rows_E], Wk[rows_O]], axis=0)
        # [128, qk, kt, 256]: per-partition contiguous 4KB per q/k half
        wqkt = np.stack(
            [m.T.reshape(KT, 128, 256).transpose(1, 0, 2) for m in (wq, wk)],
            axis=1)
        wqk_g.append(np.ascontiguousarray(wqkt.astype(nd)))

        rows_v = [h * HD + d for h in heads for d in range(HD)]
        wvt = Wv[rows_v].T.reshape(KT, 128, 256).transpose(1, 0, 2)
        wvt_g.append(np.ascontiguousarray(wvt.astype(nd)))

        wot = wo[:, rows_v].T.reshape(2, 128, 1024).transpose(1, 0, 2)
        wot_g.append(np.ascontiguousarray(wot.astype(nd)))

    in_maps = []
    for c in range(N_CORES):
        b, g = c // 4, c % 4
        in_maps.append({
            "xt": xts[b], "wqkt": wqk_g[g], "wvt": wvt_g[g], "wot": wot_g[g],
            "cosA": cosA, "sinA": sinA, "tril2": tril2,
        })
    return in_maps


def _host_fallback(x, freqs_cis, mask, wqkv, wo):
    """Generic-mask reference path (numpy, chunked over heads)."""
    x = np.asarray(x, np.float64)
    fc = np.asarray(freqs_cis, np.float64)
    m = np.asarray(mask, bool)[0, 0]
    wqkv64 = np.asarray(wqkv, np.float64)
    wo64 = np.asarray(wo, np.float64)
    qkv = x @ wqkv64.T
    q, k, v = np.split(qkv, 3, axis=-1)
    q = q.reshape(B, S, N_HEAD, HD)
    k = k.reshape(B, S, N_HEAD, HD)
    v = v.reshape(B, S, N_HEAD, HD)

    def rope(t):
        ts = t.reshape(*t.shape[:-1], HD // 2, 2)
        cr = fc[None, :, None, :, 0]
        ci = fc[None, :, None, :, 1]
        xr, xi = ts[..., 0], ts[..., 1]
        return np.stack([xr * cr - xi * ci, xi * cr + xr * ci],
                        axis=-1).reshape(t.shape)

    q, k = rope(q), rope(k)
    out = np.zeros((B, S, DIM), np.float64)
    for h in range(N_HEAD):
        sc = np.einsum("bqd,bkd->bqk", q[:, :, h], k[:, :, h]) * (HD ** -0.5)
        sc = np.where(m[None], sc, -np.inf)
        sc -= sc.max(axis=-1, keepdims=True)
        e = np.exp(sc)
        attn = e / e.sum(axis=-1, keepdims=True)
        y = np.einsum("bqk,bkd->bqd", attn, v[:, :, h])
        out += y @ wo64[:, h * HD:(h + 1) * HD].T
    return out.astype(np.float32)


def kernel(x, freqs_cis, mask, wqkv, wo):
    mask_sq = np.asarray(mask, bool)[0, 0]
    if np.array_equal(mask_sq, np.tril(np.ones((S, S), bool))):
        causal = True
    elif mask_sq.all():
        causal = False
    else:
        return _host_fallback(x, freqs_cis, mask, wqkv, wo)

    # bf16 operands are plenty for genuine rotary tables (cos^2+sin^2=1);
    # free-form freqs widen the logit range beyond bf16 comfort, so take the
    # exact host path for that (not expected in practice).
    fc = np.asarray(freqs_cis, np.float32)
    if not np.allclose(fc[..., 0] ** 2 + fc[..., 1] ** 2, 1.0, atol=0.2):
        return _host_fallback(x, freqs_cis, mask, wqkv, wo)
    md = BF16
    nc = _get_program(causal, md)
    in_maps = _host_prep(x, freqs_cis, wqkv, wo, md)
    res = run_bass_kernel_spmd(nc, in_maps, core_ids=list(range(N_CORES)))

    out = np.zeros((B, S, DIM), np.float32)
    for c in range(N_CORES):
        b = c // 4
        out[b] += res.results[c]["outp"].reshape(DIM, S).T.astype(np.float32)
    return out

